# revision 13
# baseline (speedup 1.0000x reference)
"""GCN-GRU encoder (DCRNN-style) on 8 TRN2 NeuronCores, data-parallel over B.

v3: batch-stacked layout. Per core (B_loc=4 = 2 pairs):
  HW[p]   [128, 512] bf16  GRU state, row = bi*64 + f, col = node i
  HN_s[p] [128, 512] fp8   node-major 16*h, col = j*128 + bi*64 + f (agg lhsT)
  CN_s[p] same for r*h (bf16, or fp8 when CAND_FP8)
  gt[k][j] [128, 512] bf16 G[k].T j-tile (agg rhs)
  tAB[k]  [128, 512] bf16  per-pair hop-k aggregates, row = bi*64 + f
  ax[bi]  [70, 512]        k2 aggregate rows 0:64 + 6 static x-aggregate rows
Gates: per-batch matmul -> zrt [128 = z|r, 512] PSUM; 2 sigmoids per batch
write z into zS[bi*64:+64] and r into rS (batch-stacked [128,512] tiles).
GRU elementwise entirely in [128, 512] bf16 (full 128 DVE lanes).
Transposes: 4x 128x128 PE transposes per [128,512] tile.
"""
import numpy as np
import ml_dtypes

import concourse.bass as bass
import concourse.tile as tile
from concourse import mybir
from concourse.bass_utils import run_bass_kernel_spmd

dt = mybir.dt
AF = mybir.ActivationFunctionType
ALU = mybir.AluOpType

B, T, N, C, H, K = 32, 12, 512, 2, 64, 3
NCORES = 8
BL = B // NCORES          # 4 batches per core
NT = N // 128             # 4 partition tiles of the node dim
P = C + H                 # 66
BF = dt.bfloat16
NPBF = ml_dtypes.bfloat16
NPF8 = ml_dtypes.float8_e4m3fn

_waitsplit_ctr = [0]

# tuning knobs (module-level so experiments can flip them before build)
CAND_FP8 = False      # candidate aggregation in fp8 DoubleRow
OFF_TICKS = 1         # phase offset between the two batch pairs


def _split_excess_waits(nc, max_waits=1):
    """This walrus build allows only `max_waits` semaphore waits per
    instruction; hoist the excess onto preceding same-engine NoOps."""
    for f in nc.m.functions:
        for blk in f.blocks:
            new = []
            for inst in blk.instructions:
                si = inst.sync_info
                if si is not None and len(si.on_wait) > max_waits:
                    waits = list(si.on_wait)
                    head, tail = waits[:-max_waits], waits[-max_waits:]
                    for s in range(0, len(head), max_waits):
                        nop = mybir.InstNoOp(
                            name=f"I-waitsplit-{_waitsplit_ctr[0]}", ins=[], outs=[])
                        _waitsplit_ctr[0] += 1
                        nop.engine = inst.engine
                        nop.sync_info = mybir.SyncInfo(
                            on_wait=list(head[s:s + max_waits]), on_update=[])
                        new.append(nop)
                    inst.sync_info = mybir.SyncInfo(
                        on_wait=list(tail), on_update=list(si.on_update))
                new.append(inst)
            blk.instructions[:] = new


def _build_nc(debug=False, h0_zero=False):
    nc = bass.Bass()
    f32 = dt.float32
    GT_d = nc.declare_dram_parameter("GT", [NT, 128, K * N], BF,
                                     isOutput=False)
    XN_d = nc.declare_dram_parameter("XN", [N, BL * T * C], BF, isOutput=False)
    HN0_d = nc.declare_dram_parameter("HN0", [2, 128, N],
                                      dt.float8e4, isOutput=False)
    HW0_d = nc.declare_dram_parameter("HW0", [2, 128, N], BF, isOutput=False)
    # weight blob cols: wg0 0:128 | wg1 128:256 | wgx2 256:384 |
    # wu0 384:448 | wu1 448:512 | wux2 512:576 | eye128 576:704 |
    # bg 704:705 | bu 705:706   (biases bf16)
    WB_d = nc.declare_dram_parameter("WB", [128, 706], BF, isOutput=False)
    F8 = dt.float8e4
    GT8_d = nc.declare_dram_parameter("GT8", [128, (NT // 2) * K * 2 * N], F8,
                                      isOutput=False)
    HOUT_d = nc.declare_dram_parameter("HOUT", [2, 128, N], BF, isOutput=True)
    dbg = {}
    if debug:
        for nm, shp in [("DXA", [102, T * N]), ("DT01", [128, N]),
                        ("DAX", [70, N]), ("DZS", [128, N]),
                        ("DRS", [128, N]), ("DCH", [128, N]),
                        ("DHCS", [128, N]), ("DHW1", [128, N]),
                        ("DHN", [128, N]), ("DCN", [128, N])]:
            dbg[nm] = nc.declare_dram_parameter(nm, shp, BF, isOutput=True)
    XAS_d = nc.dram_tensor("XAS_scratch", [K, BL * T * C, N], BF)

    with tile.TileContext(nc) as tc:
        with tc.tile_pool(name="const", bufs=1) as cst, \
             tc.tile_pool(name="t01s", bufs=2) as t01p, \
             tc.tile_pool(name="sb", bufs=2) as sbp, \
             tc.tile_pool(name="aggps", bufs=4, space="PSUM") as aggps, \
             tc.tile_pool(name="zrps", bufs=2, space="PSUM") as zrps, \
             tc.tile_pool(name="trps", bufs=2, space="PSUM") as trps:

            def load(shape, src_ap, tag, dtype=BF):
                d = cst.tile(shape, dtype, tag=tag)
                nc.sync.dma_start(d[:], src_ap)
                return d

            # ---- constants / inputs (gt halves around xn so the XA
            # precompute can start as early as possible) ----
            gtall = cst.tile([128, NT * K * N], BF, tag="gtall")
            half = NT * K * N // 2
            nc.sync.dma_start(
                gtall[:, 0:half].rearrange("p (j ki) -> p j ki", j=NT // 2),
                GT_d[0:NT // 2].rearrange("j p ki -> p j ki"))
            xnall = cst.tile([128, NT * BL * T * C], BF, tag="xnall")
            nc.sync.dma_start(
                xnall[:].rearrange("p (j q) -> p j q", j=NT),
                XN_d.rearrange("(j p) q -> p j q", j=NT))
            xn = [xnall[:, j * BL * T * C:(j + 1) * BL * T * C]
                  for j in range(NT)]
            nc.sync.dma_start(
                gtall[:, half:].rearrange("p (j ki) -> p j ki", j=NT // 2),
                GT_d[NT // 2:].rearrange("j p ki -> p j ki"))
            gt = [[gtall[:, (j * K + k) * N:(j * K + k + 1) * N]
                   for j in range(NT)] for k in range(K)]
            gt8all = cst.tile([128, (NT // 2) * K * 2 * N], F8, tag="gt8all")
            nc.sync.dma_start(gt8all[:], GT8_d[:])
            # gt8[jp][k]: [128, 2, N] fp8 double-row operand
            gt8 = [[gt8all[:, ((jp * K + k) * 2) * N:
                           ((jp * K + k) * 2 + 2) * N].rearrange(
                               "p (kt i) -> p kt i", kt=2)
                    for k in range(K)] for jp in range(NT // 2)]

            HN_s = [None, None]
            HW = [None, None]
            if not h0_zero:
                for p in range(2):
                    hn0 = sbp.tile([128, N], F8, tag=f"hn{p}",
                                   name=f"hn0{p}")
                    nc.sync.dma_start(hn0[:], HN0_d[p])
                    HN_s[p] = hn0
                    hw0 = sbp.tile([128, N], BF, tag=f"hw{p}",
                                   name=f"hw0{p}")
                    nc.sync.dma_start(hw0[:], HW0_d[p])
                    HW[p] = hw0
            wb = load([128, 706], WB_d[:], "wb")
            wg0 = wb[:, 0:128]
            wg1 = wb[:, 128:256]
            wgx2 = wb[0:70, 256:384]
            wu0 = wb[:, 384:448]
            wu1 = wb[:, 448:512]
            wux2 = wb[0:70, 512:576]
            eye = wb[:, 576:704]
            bg = wb[:, 704:705]
            bu = wb[0:64, 705:706]

            # static x-aggregates: row b*32 + k*2 + c, col t*512+i
            XA24 = cst.tile([(BL - 1) * 32 + C * K, T * N], BF, tag="xa24")


            # ---- x aggregation precompute (emitted after the first
            # h-aggregations so PE starts on loop work immediately) ----
            def xa_precompute():
                for k in range(K):
                    ps = aggps.tile([BL * T * C, N], f32, tag="agg")
                    for j in range(NT):
                        nc.tensor.matmul(ps[:], xn[j], gt[k][j],
                                         start=(j == 0), stop=(j == NT - 1))
                    xas = sbp.tile([BL * T * C, N], BF, tag=f"xas{k}",
                                   name=f"xas{k}")
                    nc.vector.tensor_copy(xas[:], ps[:])
                    for b in range(BL):
                        eng = nc.sync if b < 3 else nc.gpsimd
                        for c in range(C):
                            row = b * 32 + k * 2 + c
                            eng.dma_start(
                                XA24[row:row + 1, :],
                                xas[b * 24 + c * T:b * 24 + (c + 1) * T, :])

            # ---- per-step phase bodies ----
            # k2 drains rotate on DVE (keeps Act free for activations)
            def drain(ci, dst_ap, src_ap):
                i_ = nc.vector.tensor_copy(dst_ap, src_ap)
                i_.ins.bass_priority = -20

            st = [dict(), dict()]

            def xcopy(p, t, ax2):
                # static x rows: off the critical path, on idle DMA engines
                for bi in range(2):
                    b = 2 * p + bi
                    nc.gpsimd.dma_start(ax2[bi][64:70, :],
                                        XA24[b * 32:b * 32 + 6,
                                             t * N:(t + 1) * N])

            def agg_zero(p, t, dtag):
                """t=0 with h==0: aggregates are all zero; only the ax
                tiles (x rows + zeroed k2 rows) are materialized."""
                ax2 = []
                for bi in range(2):
                    ax = t01p.tile([70, N], BF, tag=f"ax{dtag}{p}{bi}",
                                   name=f"axz{dtag}{p}{bi}")
                    nc.vector.memset(ax[0:64, :], 0.0)
                    ax2.append(ax)
                xcopy(p, t, ax2)
                return None, ax2

            def agg(p, t, src, dtag, skip_xcopy=False, fp8=False):
                """3-hop aggregation of node-major src; k0/k1 -> straight
                pair tiles tA/tB, k2+x -> per-batch [70, 512] ax tiles.
                fp8: double-row matmuls (gate side; src is the fp8 HN)."""
                ax2 = []
                for bi in range(2):
                    ax = t01p.tile([70, N], BF, tag=f"ax{dtag}{p}{bi}",
                                   name=f"ax{dtag}{p}{bi}")
                    ax2.append(ax)
                if not skip_xcopy:
                    xcopy(p, t, ax2)
                psk = {}
                for k in (2, 0, 1):
                    ps = aggps.tile([128, N], f32, tag="agg")
                    if fp8:
                        for jp in range(NT // 2):
                            lhs = src[:, jp * 256:(jp + 1) * 256].rearrange(
                                "p (kt m) -> p kt m", kt=2)
                            nc.tensor.matmul(
                                ps[:], lhs, gt8[jp][k],
                                start=(jp == 0), stop=(jp == NT // 2 - 1),
                                perf_mode=mybir.MatmulPerfMode.DoubleRow)
                    else:
                        for j in range(NT):
                            nc.tensor.matmul(
                                ps[:], src[:, j * 128:(j + 1) * 128],
                                gt[k][j],
                                start=(j == 0), stop=(j == NT - 1))
                    psk[k] = ps
                    if k == 2:
                        for bi in range(2):
                            drain(2 * p + bi, ax2[bi][0:64, :],
                                  ps[bi * 64:bi * 64 + 64, :])
                tAB = []
                for k in range(2):
                    tt = t01p.tile([128, N], BF, tag=f"{dtag}{p}{k}")
                    # spread the two big hop drains: k0 -> DVE, k1 -> Act
                    if k == 0:
                        i_ = nc.vector.tensor_copy(tt[:], psk[k][:])
                    else:
                        i_ = nc.scalar.copy(tt[:], psk[k][:])
                    i_.ins.bass_priority = -20
                    tAB.append(tt)
                return tAB, ax2

            def transp(p, src, dst_tag, fp8=False):
                """batch-stacked [128, 512] bf16 -> node-major [128, 512];
                drained per half so the next agg's first matmul never
                waits the full transpose set."""
                trp = trps.tile([128, N], BF, tag="tr")
                d = sbp.tile([128, N], F8 if fp8 else BF,
                             tag=f"{dst_tag}{p}", name=f"{dst_tag}{p}")
                for j in range(NT):
                    nc.tensor.transpose(
                        trp[:, j * 128:(j + 1) * 128],
                        src[:, j * 128:(j + 1) * 128],
                        eye)
                for hh in range(2):
                    sl = slice(hh * (N // 2), (hh + 1) * (N // 2))
                    if fp8:
                        i_ = nc.vector.tensor_scalar_mul(d[:, sl], trp[:, sl],
                                                         16.0)
                    else:
                        i_ = nc.vector.tensor_copy(d[:, sl], trp[:, sl])
                    i_.ins.bass_priority = -20
                return d

            def ph_agg_gate(p, t, skip_xcopy=False):
                if h0_zero and t == 0:
                    st[p]["t01g"] = agg_zero(p, t, "tg")
                else:
                    st[p]["t01g"] = agg(p, t, HN_s[p], "tg", skip_xcopy,
                                        fp8=True)

            def ph_gate_w(p, t):
                tAB, ax2 = st[p]["t01g"]
                zS = sbp.tile([128, N], BF, tag=f"zs{p}")
                rS = sbp.tile([128, N], BF, tag=f"rs{p}")
                zrt = [zrps.tile([128, N], f32, tag="zr", name=f"zr{p}{bi}")
                       for bi in range(2)]
                if tAB is None:
                    for bi in range(2):
                        nc.tensor.matmul(zrt[bi][:], wgx2, ax2[bi][:],
                                         start=True, stop=True)
                        bs = slice(bi * 64, bi * 64 + 64)
                        nc.scalar.activation(zS[bs, :], zrt[bi][0:64, :],
                                             AF.Sigmoid, bias=bg[0:64, :])
                        nc.scalar.activation(rS[bs, :], zrt[bi][64:128, :],
                                             AF.Sigmoid, bias=bg[64:128, :])
                else:
                    for bi in range(2):
                        bs = slice(bi * 64, bi * 64 + 64)
                        nc.tensor.matmul(zrt[bi][:], wgx2, ax2[bi][:],
                                         start=True, stop=False)
                        nc.tensor.matmul(zrt[bi][:], wg0[bs, :], tAB[0][bs, :],
                                         start=False, stop=False)
                    for bi in range(2):
                        bs = slice(bi * 64, bi * 64 + 64)
                        nc.tensor.matmul(zrt[bi][:], wg1[bs, :], tAB[1][bs, :],
                                         start=False, stop=True)
                        nc.scalar.activation(zS[bs, :], zrt[bi][0:64, :],
                                             AF.Sigmoid, bias=bg[0:64, :])
                        nc.scalar.activation(rS[bs, :], zrt[bi][64:128, :],
                                             AF.Sigmoid, bias=bg[64:128, :])
                st[p]["zS"], st[p]["rS"] = zS, rS

            def ph_rt(p, t):
                if h0_zero and t == 0:
                    return
                zS, rS = st[p]["zS"], st[p]["rS"]
                ch = sbp.tile([128, N], BF, tag=f"ch{p}")
                nc.vector.tensor_tensor(ch[:], rS[:], HW[p][:], ALU.mult)
                st[p]["CN"] = transp(p, ch, "cn", fp8=CAND_FP8)
                st[p]["ch_dbg"] = ch
                # u = h - z*h = (1-z)*h, off the critical path
                zh = sbp.tile([128, N], BF, tag=f"tmp{p}", bufs=2,
                              name=f"zh{p}")
                nc.gpsimd.tensor_tensor(zh[:], zS[:], HW[p][:], ALU.mult)
                u = sbp.tile([128, N], BF, tag=f"u{p}")
                nc.gpsimd.tensor_tensor(u[:], HW[p][:], zh[:], ALU.subtract)
                st[p]["u"] = u

            def ph_agg_cand(p, t):
                if h0_zero and t == 0:
                    st[p]["t01u"] = agg_zero(p, t, "tu")
                else:
                    st[p]["t01u"] = agg(p, t, st[p]["CN"], "tu",
                                        fp8=CAND_FP8)

            def ph_upd_w(p, t):
                tAB, ax2 = st[p]["t01u"]
                hcs = sbp.tile([128, N], BF, tag=f"hcs{p}")
                hct = [zrps.tile([128, N], f32, tag="zr", name=f"hc{p}{bi}")
                       for bi in range(2)]
                if tAB is None:
                    for bi in range(2):
                        bs = slice(bi * 64, bi * 64 + 64)
                        nc.tensor.matmul(hct[bi][0:64, :], wux2, ax2[bi][:],
                                         start=True, stop=True)
                        nc.scalar.activation(hcs[bs, :], hct[bi][0:64, :],
                                             AF.Tanh, bias=bu)
                else:
                    for bi in range(2):
                        bs = slice(bi * 64, bi * 64 + 64)
                        nc.tensor.matmul(hct[bi][0:64, :], wux2, ax2[bi][:],
                                         start=True, stop=False)
                        nc.tensor.matmul(hct[bi][0:64, :], wu0[bs, :],
                                         tAB[0][bs, :], start=False,
                                         stop=False)
                    for bi in range(2):
                        bs = slice(bi * 64, bi * 64 + 64)
                        nc.tensor.matmul(hct[bi][0:64, :], wu1[bs, :],
                                         tAB[1][bs, :], start=False, stop=True)
                        nc.scalar.activation(hcs[bs, :], hct[bi][0:64, :],
                                             AF.Tanh, bias=bu)
                st[p]["hcs"] = hcs

            def ph_update(p, t):
                zS, hcs = st[p]["zS"], st[p]["hcs"]
                hnew = sbp.tile([128, N], BF, tag=f"hw{p}")
                if h0_zero and t == 0:
                    # h1 = z * hc (since h == 0)
                    nc.vector.tensor_tensor(hnew[:], zS[:], hcs[:], ALU.mult)
                else:
                    u = st[p]["u"]
                    v = sbp.tile([128, N], BF, tag=f"tmp{p}", bufs=2)
                    if t < T - 1:
                        nc.vector.tensor_tensor(v[:], zS[:], hcs[:], ALU.mult)
                        nc.vector.tensor_tensor(hnew[:], u[:], v[:], ALU.add)
                    else:
                        # final step: halves, so the output DMA overlaps
                        for hh in range(2):
                            sl = slice(hh * (N // 2), (hh + 1) * (N // 2))
                            nc.vector.tensor_tensor(v[:, sl], zS[:, sl],
                                                    hcs[:, sl], ALU.mult)
                            nc.vector.tensor_tensor(hnew[:, sl], u[:, sl],
                                                    v[:, sl], ALU.add)
                            nc.sync.dma_start(HOUT_d[p, :, sl],
                                              hnew[:, sl])
                HW[p] = hnew
                if t < T - 1:
                    HN_s[p] = transp(p, hnew, "hn2", fp8=True)

            def dump(nm, ap):
                if debug:
                    nc.sync.dma_start(dbg[nm][0:ap.shape[0]], ap)

            def ph_dbg(p, t):
                import os
                if not debug or p != 0 or t != int(os.environ.get("DBG_T", "0")):
                    return
                dump("DXA", XA24[:])
                dump("DT01", st[0]["t01g"][0][0][:])
                dump("DAX", st[0]["t01g"][1][0][:])
                dump("DZS", st[0]["zS"][:])
                dump("DRS", st[0]["rS"][:])
                dump("DCH", st[0]["ch_dbg"][:])
                dump("DHCS", st[0]["hcs"][:])
                dump("DHW1", HW[0][:])
                dump("DHN", HN_s[0][:])
                dump("DCN", st[0]["CN"][:])

            PHASES = [ph_agg_gate, ph_gate_w, ph_rt, ph_agg_cand,
                      ph_upd_w, ph_update, ph_dbg]
            NPH = len(PHASES)
            OFF = OFF_TICKS
            xa_precompute()
            for tick in range(NPH * T + OFF):
                for p in range(2):
                    local = tick - OFF * p
                    if 0 <= local < NPH * T:
                        t, ph = divmod(local, NPH)
                        PHASES[ph](p, t)

    _split_excess_waits(nc, max_waits=1)
    return nc


_NC_CACHE = {}


def _get_nc(debug=False, h0_zero=False):
    key = f"nc{debug}{h0_zero}{CAND_FP8}{OFF_TICKS}"
    if key not in _NC_CACHE:
        _NC_CACHE[key] = _build_nc(debug, h0_zero)
    return _NC_CACHE[key]


def _host_prep(G, x_seq, init_h, W_gate, b_gate, W_update, b_update):
    f32 = np.float32
    GTf = np.asarray(G, np.float32).transpose(0, 2, 1)         # [k, jn, i]
    GT = GTf.reshape(K, NT, 128, N).transpose(1, 2, 0, 3)      # [j, p, k, i]
    GT = np.ascontiguousarray(GT).reshape(NT, 128, K * N).astype(NPBF)
    # GT8[p, jp, k, kt, i] = fp8(64 * G_k[i, (2jp+kt)*128+p])
    G8 = (64.0 * GTf).reshape(K, NT // 2, 2, 128, N)           # [k, jp, kt, p, i]
    G8 = G8.transpose(3, 1, 0, 2, 4)                           # [p, jp, k, kt, i]
    GT8 = np.ascontiguousarray(G8).reshape(
        128, (NT // 2) * K * 2 * N).astype(NPF8)
    WG3 = np.asarray(W_gate, f32).reshape(K, P, 2 * H)
    WU3 = np.asarray(W_update, f32).reshape(K, P, H)
    WG0 = np.concatenate([WG3[0, C:, :]] * 2, axis=0)
    WG1 = np.concatenate([WG3[1, C:, :]] * 2, axis=0)
    WU0 = np.concatenate([WU3[0, C:, :]] * 2, axis=0)
    WU1 = np.concatenate([WU3[1, C:, :]] * 2, axis=0)
    # x-block rows (k,c): row k*2+c = W[k, c, :]
    xg = WG3[:, :C, :].reshape(K * C, 2 * H)
    xu = WU3[:, :C, :].reshape(K * C, H)
    WGX2 = np.concatenate([WG3[2, C:, :], xg], axis=0)
    WUX2 = np.concatenate([WU3[2, C:, :], xu], axis=0)
    WB = np.zeros((128, 706), f32)
    WB[:, 0:128] = WG0 / 1024.0
    WB[:, 128:256] = WG1 / 1024.0
    WB[0:70, 256:384] = WGX2
    WB[0:64, 256:384] = WGX2[0:64] / 1024.0
    usc = 1024.0 if CAND_FP8 else 1.0
    WB[:, 384:448] = WU0 / usc
    WB[:, 448:512] = WU1 / usc
    WB[0:70, 512:576] = WUX2
    WB[0:64, 512:576] = WUX2[0:64] / usc
    WB[:, 576:704] = np.eye(128, dtype=f32)
    WB[:, 704] = np.asarray(b_gate, f32)
    WB[0:64, 705] = np.asarray(b_update, f32)
    shared = {
        "GT": GT,
        "WB": WB.astype(NPBF),
        "GT8": GT8,
    }
    x_seq = np.asarray(x_seq, f32)
    init_h = np.asarray(init_h, f32)
    in_maps = []
    for c in range(NCORES):
        b0 = c * BL
        xs = x_seq[b0:b0 + BL]                     # [4, 12, 512, 2]
        h0 = init_h[b0:b0 + BL]                    # [4, 512, 64]
        m = dict(shared)
        # XN cols (b, c, t)
        m["XN"] = np.ascontiguousarray(
            xs.transpose(2, 0, 3, 1)).reshape(N, BL * T * C).astype(NPBF)
        # HN0[p][n_loc, j*128 + b*64 + f] = h0[2p+b, j*128+n_loc, f]
        hn = h0.reshape(2, 2, NT, 128, H)          # [p, b, j, n, f]
        m["HN0"] = (16.0 * np.ascontiguousarray(
            hn.transpose(0, 3, 2, 1, 4)).reshape(2, 128, N)).astype(NPF8)
        # HW0[p][b*64 + f, i] = h0[2p+b, i, f]
        hw = h0.reshape(2, 2, N, H)                # [p, b, i, f]
        m["HW0"] = np.ascontiguousarray(
            hw.transpose(0, 1, 3, 2)).reshape(2, 128, N).astype(NPBF)
        in_maps.append(m)
    return in_maps


def _run(inputs, trace=False, debug=False):
    h0_zero = not np.any(np.asarray(inputs["init_h"]))
    nc = _get_nc(debug, h0_zero)
    in_maps = _host_prep(**inputs)
    res = run_bass_kernel_spmd(nc, in_maps, list(range(NCORES)), trace=trace)
    outs = []
    for c in range(NCORES):
        hout = np.asarray(res.results[c]["HOUT"], dtype=np.float32)
        # [2, 128, 512] -> [4, 512, 64]
        hout = hout.reshape(2, 2, H, N).transpose(0, 1, 3, 2).reshape(
            BL, N, H)
        outs.append(hout)
    full = np.concatenate(outs, axis=0).astype(np.float32)
    return full, res


def kernel(G, x_seq, init_h, W_gate, b_gate, W_update, b_update):
    full, _ = _run(dict(G=G, x_seq=x_seq, init_h=init_h, W_gate=W_gate,
                        b_gate=b_gate, W_update=W_update, b_update=b_update))
    return full


# revision 14
# speedup vs baseline: 1.0392x; 1.0392x over previous
"""GCN-GRU encoder (DCRNN-style) on 8 TRN2 NeuronCores, data-parallel over B.

v3: batch-stacked layout. Per core (B_loc=4 = 2 pairs):
  HW[p]   [128, 512] bf16  GRU state, row = bi*64 + f, col = node i
  HN_s[p] [128, 512] fp8   node-major 16*h, col = j*128 + bi*64 + f (agg lhsT)
  CN_s[p] same for r*h (bf16, or fp8 when CAND_FP8)
  gt[k][j] [128, 512] bf16 G[k].T j-tile (agg rhs)
  tAB[k]  [128, 512] bf16  per-pair hop-k aggregates, row = bi*64 + f
  ax[bi]  [70, 512]        k2 aggregate rows 0:64 + 6 static x-aggregate rows
Gates: per-batch matmul -> zrt [128 = z|r, 512] PSUM; 2 sigmoids per batch
write z into zS[bi*64:+64] and r into rS (batch-stacked [128,512] tiles).
GRU elementwise entirely in [128, 512] bf16 (full 128 DVE lanes).
Transposes: 4x 128x128 PE transposes per [128,512] tile.
"""
import numpy as np
import ml_dtypes

import concourse.bass as bass
import concourse.tile as tile
from concourse import mybir
from concourse.bass_utils import run_bass_kernel_spmd

dt = mybir.dt
AF = mybir.ActivationFunctionType
ALU = mybir.AluOpType

B, T, N, C, H, K = 32, 12, 512, 2, 64, 3
NCORES = 8
BL = B // NCORES          # 4 batches per core
NT = N // 128             # 4 partition tiles of the node dim
P = C + H                 # 66
BF = dt.bfloat16
NPBF = ml_dtypes.bfloat16
NPF8 = ml_dtypes.float8_e4m3fn

_waitsplit_ctr = [0]

# tuning knobs (module-level so experiments can flip them before build)
CAND_FP8 = False      # candidate aggregation in fp8 DoubleRow
OFF_TICKS = 1         # phase offset between the two batch pairs


def _split_excess_waits(nc, max_waits=1):
    """This walrus build allows only `max_waits` semaphore waits per
    instruction; hoist the excess onto preceding same-engine NoOps."""
    for f in nc.m.functions:
        for blk in f.blocks:
            new = []
            for inst in blk.instructions:
                si = inst.sync_info
                if si is not None and len(si.on_wait) > max_waits:
                    waits = list(si.on_wait)
                    head, tail = waits[:-max_waits], waits[-max_waits:]
                    for s in range(0, len(head), max_waits):
                        nop = mybir.InstNoOp(
                            name=f"I-waitsplit-{_waitsplit_ctr[0]}", ins=[], outs=[])
                        _waitsplit_ctr[0] += 1
                        nop.engine = inst.engine
                        nop.sync_info = mybir.SyncInfo(
                            on_wait=list(head[s:s + max_waits]), on_update=[])
                        new.append(nop)
                    inst.sync_info = mybir.SyncInfo(
                        on_wait=list(tail), on_update=list(si.on_update))
                new.append(inst)
            blk.instructions[:] = new


def _build_nc(debug=False, h0_zero=False):
    nc = bass.Bass()
    f32 = dt.float32
    GT_d = nc.declare_dram_parameter("GT", [NT, 128, K * N], BF,
                                     isOutput=False)
    XN_d = nc.declare_dram_parameter("XN", [N, BL * T * C], BF, isOutput=False)
    HN0_d = nc.declare_dram_parameter("HN0", [2, 128, N],
                                      dt.float8e4, isOutput=False)
    HW0_d = nc.declare_dram_parameter("HW0", [2, 128, N], BF, isOutput=False)
    # weight blob cols: wg0 0:128 | wg1 128:256 | wgx2 256:384 |
    # wu0 384:448 | wu1 448:512 | wux2 512:576 | eye128 576:704 |
    # bg 704:705 | bu 705:706   (biases bf16)
    WB_d = nc.declare_dram_parameter("WB", [128, 706], BF, isOutput=False)
    F8 = dt.float8e4
    GT8_d = nc.declare_dram_parameter("GT8", [128, (NT // 2) * K * 2 * N], F8,
                                      isOutput=False)
    HOUT_d = nc.declare_dram_parameter("HOUT", [2, 128, N], BF, isOutput=True)
    dbg = {}
    if debug:
        for nm, shp in [("DXA", [102, T * N]), ("DT01", [128, N]),
                        ("DAX", [70, N]), ("DZS", [128, N]),
                        ("DRS", [128, N]), ("DCH", [128, N]),
                        ("DHCS", [128, N]), ("DHW1", [128, N]),
                        ("DHN", [128, N]), ("DCN", [128, N])]:
            dbg[nm] = nc.declare_dram_parameter(nm, shp, BF, isOutput=True)
    XAS_d = nc.dram_tensor("XAS_scratch", [K, BL * T * C, N], BF)

    with tile.TileContext(nc) as tc:
        with tc.tile_pool(name="const", bufs=1) as cst, \
             tc.tile_pool(name="t01s", bufs=2) as t01p, \
             tc.tile_pool(name="sb", bufs=2) as sbp, \
             tc.tile_pool(name="aggps", bufs=4, space="PSUM") as aggps, \
             tc.tile_pool(name="zrps", bufs=2, space="PSUM") as zrps, \
             tc.tile_pool(name="trps", bufs=2, space="PSUM") as trps:

            def load(shape, src_ap, tag, dtype=BF):
                d = cst.tile(shape, dtype, tag=tag)
                nc.sync.dma_start(d[:], src_ap)
                return d

            # ---- constants / inputs (gt halves around xn so the XA
            # precompute can start as early as possible) ----
            gtall = cst.tile([128, NT * K * N], BF, tag="gtall")
            half = NT * K * N // 2
            nc.sync.dma_start(
                gtall[:, 0:half].rearrange("p (j ki) -> p j ki", j=NT // 2),
                GT_d[0:NT // 2].rearrange("j p ki -> p j ki"))
            xnall = cst.tile([128, NT * BL * T * C], BF, tag="xnall")
            nc.sync.dma_start(
                xnall[:].rearrange("p (j q) -> p j q", j=NT),
                XN_d.rearrange("(j p) q -> p j q", j=NT))
            xn = [xnall[:, j * BL * T * C:(j + 1) * BL * T * C]
                  for j in range(NT)]
            nc.sync.dma_start(
                gtall[:, half:].rearrange("p (j ki) -> p j ki", j=NT // 2),
                GT_d[NT // 2:].rearrange("j p ki -> p j ki"))
            gt = [[gtall[:, (j * K + k) * N:(j * K + k + 1) * N]
                   for j in range(NT)] for k in range(K)]
            gt8all = cst.tile([128, (NT // 2) * K * 2 * N], F8, tag="gt8all")
            nc.sync.dma_start(gt8all[:], GT8_d[:])
            # gt8[jp][k]: [128, 2, N] fp8 double-row operand
            gt8 = [[gt8all[:, ((jp * K + k) * 2) * N:
                           ((jp * K + k) * 2 + 2) * N].rearrange(
                               "p (kt i) -> p kt i", kt=2)
                    for k in range(K)] for jp in range(NT // 2)]

            HN_s = [None, None]
            HW = [None, None]
            if not h0_zero:
                for p in range(2):
                    hn0 = sbp.tile([128, N], F8, tag=f"hn{p}",
                                   name=f"hn0{p}")
                    nc.sync.dma_start(hn0[:], HN0_d[p])
                    HN_s[p] = hn0
                    hw0 = sbp.tile([128, N], BF, tag=f"hw{p}",
                                   name=f"hw0{p}")
                    nc.sync.dma_start(hw0[:], HW0_d[p])
                    HW[p] = hw0
            wb = load([128, 706], WB_d[:], "wb")
            wg0 = wb[:, 0:128]
            wg1 = wb[:, 128:256]
            wgx2 = wb[0:70, 256:384]
            wu0 = wb[:, 384:448]
            wu1 = wb[:, 448:512]
            wux2 = wb[0:70, 512:576]
            eye = wb[:, 576:704]
            bg = wb[:, 704:705]
            bu = wb[0:64, 705:706]

            # static x-aggregates: row b*32 + k*2 + c, col t*512+i
            XA24 = cst.tile([(BL - 1) * 32 + C * K, T * N], BF, tag="xa24")


            # ---- x aggregation precompute (emitted after the first
            # h-aggregations so PE starts on loop work immediately) ----
            def xa_precompute():
                for k in range(K):
                    ps = aggps.tile([BL * T * C, N], f32, tag="agg")
                    for j in range(NT):
                        nc.tensor.matmul(ps[:], xn[j], gt[k][j],
                                         start=(j == 0), stop=(j == NT - 1))
                    xas = sbp.tile([BL * T * C, N], BF, tag=f"xas{k}",
                                   name=f"xas{k}")
                    nc.vector.tensor_copy(xas[:], ps[:])
                    for b in range(BL):
                        eng = nc.sync if b < 3 else nc.gpsimd
                        for c in range(C):
                            row = b * 32 + k * 2 + c
                            eng.dma_start(
                                XA24[row:row + 1, :],
                                xas[b * 24 + c * T:b * 24 + (c + 1) * T, :])

            # ---- per-step phase bodies ----
            # k2 drains rotate on DVE (keeps Act free for activations)
            def drain(ci, dst_ap, src_ap):
                i_ = nc.vector.tensor_copy(dst_ap, src_ap)
                i_.ins.bass_priority = -20

            st = [dict(), dict()]

            def xcopy(p, t, ax2):
                for bi in range(2):
                    b = 2 * p + bi
                    i_ = nc.vector.tensor_copy(ax2[bi][64:70, :],
                                               XA24[b * 32:b * 32 + 6,
                                                    t * N:(t + 1) * N])
                    i_.ins.bass_priority = -20

            def agg_zero(p, t, dtag):
                """t=0 with h==0: aggregates are all zero; only the ax
                tiles (x rows + zeroed k2 rows) are materialized."""
                ax2 = []
                for bi in range(2):
                    ax = t01p.tile([70, N], BF, tag=f"ax{dtag}{p}{bi}",
                                   name=f"axz{dtag}{p}{bi}")
                    nc.vector.memset(ax[0:64, :], 0.0)
                    ax2.append(ax)
                xcopy(p, t, ax2)
                return None, ax2

            def agg(p, t, src, dtag, skip_xcopy=False, fp8=False):
                """3-hop aggregation of node-major src; k0/k1 -> straight
                pair tiles tA/tB, k2+x -> per-batch [70, 512] ax tiles.
                fp8: double-row matmuls (gate side; src is the fp8 HN)."""
                ax2 = []
                for bi in range(2):
                    ax = t01p.tile([70, N], BF, tag=f"ax{dtag}{p}{bi}",
                                   name=f"ax{dtag}{p}{bi}")
                    ax2.append(ax)
                if not skip_xcopy:
                    xcopy(p, t, ax2)
                psk = {}
                for k in (2, 0, 1):
                    ps = aggps.tile([128, N], f32, tag="agg")
                    if fp8:
                        for jp in range(NT // 2):
                            lhs = src[:, jp * 256:(jp + 1) * 256].rearrange(
                                "p (kt m) -> p kt m", kt=2)
                            nc.tensor.matmul(
                                ps[:], lhs, gt8[jp][k],
                                start=(jp == 0), stop=(jp == NT // 2 - 1),
                                perf_mode=mybir.MatmulPerfMode.DoubleRow)
                    else:
                        for j in range(NT):
                            nc.tensor.matmul(
                                ps[:], src[:, j * 128:(j + 1) * 128],
                                gt[k][j],
                                start=(j == 0), stop=(j == NT - 1))
                    psk[k] = ps
                    if k == 2:
                        for bi in range(2):
                            drain(2 * p + bi, ax2[bi][0:64, :],
                                  ps[bi * 64:bi * 64 + 64, :])
                tAB = []
                for k in range(2):
                    tt = t01p.tile([128, N], BF, tag=f"{dtag}{p}{k}")
                    # spread the two big hop drains: k0 -> DVE, k1 -> Act
                    if k == 0:
                        i_ = nc.vector.tensor_copy(tt[:], psk[k][:])
                    else:
                        i_ = nc.scalar.copy(tt[:], psk[k][:])
                    i_.ins.bass_priority = -20
                    tAB.append(tt)
                return tAB, ax2

            def transp(p, src, dst_tag, fp8=False):
                """batch-stacked [128, 512] bf16 -> node-major [128, 512];
                drained per half so the next agg's first matmul never
                waits the full transpose set."""
                trp = trps.tile([128, N], BF, tag="tr")
                d = sbp.tile([128, N], F8 if fp8 else BF,
                             tag=f"{dst_tag}{p}", name=f"{dst_tag}{p}")
                for j in range(NT):
                    nc.tensor.transpose(
                        trp[:, j * 128:(j + 1) * 128],
                        src[:, j * 128:(j + 1) * 128],
                        eye)
                for hh in range(2):
                    sl = slice(hh * (N // 2), (hh + 1) * (N // 2))
                    if fp8:
                        i_ = nc.vector.tensor_scalar_mul(d[:, sl], trp[:, sl],
                                                         16.0)
                    else:
                        i_ = nc.vector.tensor_copy(d[:, sl], trp[:, sl])
                    i_.ins.bass_priority = -20
                return d

            def ph_agg_gate(p, t, skip_xcopy=False):
                if h0_zero and t == 0:
                    st[p]["t01g"] = agg_zero(p, t, "tg")
                else:
                    st[p]["t01g"] = agg(p, t, HN_s[p], "tg", skip_xcopy,
                                        fp8=True)

            def ph_gate_w(p, t):
                tAB, ax2 = st[p]["t01g"]
                zS = sbp.tile([128, N], BF, tag=f"zs{p}")
                rS = sbp.tile([128, N], BF, tag=f"rs{p}")
                zrt = [zrps.tile([128, N], f32, tag="zr", name=f"zr{p}{bi}")
                       for bi in range(2)]
                if tAB is None:
                    for bi in range(2):
                        nc.tensor.matmul(zrt[bi][:], wgx2, ax2[bi][:],
                                         start=True, stop=True)
                        bs = slice(bi * 64, bi * 64 + 64)
                        nc.scalar.activation(zS[bs, :], zrt[bi][0:64, :],
                                             AF.Sigmoid, bias=bg[0:64, :])
                        nc.scalar.activation(rS[bs, :], zrt[bi][64:128, :],
                                             AF.Sigmoid, bias=bg[64:128, :])
                else:
                    for bi in range(2):
                        bs = slice(bi * 64, bi * 64 + 64)
                        nc.tensor.matmul(zrt[bi][:], wgx2, ax2[bi][:],
                                         start=True, stop=False)
                        nc.tensor.matmul(zrt[bi][:], wg0[bs, :], tAB[0][bs, :],
                                         start=False, stop=False)
                    for bi in range(2):
                        bs = slice(bi * 64, bi * 64 + 64)
                        nc.tensor.matmul(zrt[bi][:], wg1[bs, :], tAB[1][bs, :],
                                         start=False, stop=True)
                        nc.scalar.activation(zS[bs, :], zrt[bi][0:64, :],
                                             AF.Sigmoid, bias=bg[0:64, :])
                        nc.scalar.activation(rS[bs, :], zrt[bi][64:128, :],
                                             AF.Sigmoid, bias=bg[64:128, :])
                st[p]["zS"], st[p]["rS"] = zS, rS

            def ph_rt(p, t):
                if h0_zero and t == 0:
                    return
                zS, rS = st[p]["zS"], st[p]["rS"]
                ch = sbp.tile([128, N], BF, tag=f"ch{p}")
                nc.vector.tensor_tensor(ch[:], rS[:], HW[p][:], ALU.mult)
                st[p]["CN"] = transp(p, ch, "cn", fp8=CAND_FP8)
                st[p]["ch_dbg"] = ch
                # u = h - z*h = (1-z)*h, off the critical path
                zh = sbp.tile([128, N], BF, tag=f"tmp{p}", bufs=2,
                              name=f"zh{p}")
                nc.gpsimd.tensor_tensor(zh[:], zS[:], HW[p][:], ALU.mult)
                u = sbp.tile([128, N], BF, tag=f"u{p}")
                nc.gpsimd.tensor_tensor(u[:], HW[p][:], zh[:], ALU.subtract)
                st[p]["u"] = u

            def ph_agg_cand(p, t):
                if h0_zero and t == 0:
                    st[p]["t01u"] = agg_zero(p, t, "tu")
                else:
                    st[p]["t01u"] = agg(p, t, st[p]["CN"], "tu",
                                        fp8=CAND_FP8)

            def ph_upd_w(p, t):
                tAB, ax2 = st[p]["t01u"]
                hcs = sbp.tile([128, N], BF, tag=f"hcs{p}")
                hct = [zrps.tile([128, N], f32, tag="zr", name=f"hc{p}{bi}")
                       for bi in range(2)]
                if tAB is None:
                    for bi in range(2):
                        bs = slice(bi * 64, bi * 64 + 64)
                        nc.tensor.matmul(hct[bi][0:64, :], wux2, ax2[bi][:],
                                         start=True, stop=True)
                        nc.scalar.activation(hcs[bs, :], hct[bi][0:64, :],
                                             AF.Tanh, bias=bu)
                else:
                    for bi in range(2):
                        bs = slice(bi * 64, bi * 64 + 64)
                        nc.tensor.matmul(hct[bi][0:64, :], wux2, ax2[bi][:],
                                         start=True, stop=False)
                        nc.tensor.matmul(hct[bi][0:64, :], wu0[bs, :],
                                         tAB[0][bs, :], start=False,
                                         stop=False)
                    for bi in range(2):
                        bs = slice(bi * 64, bi * 64 + 64)
                        nc.tensor.matmul(hct[bi][0:64, :], wu1[bs, :],
                                         tAB[1][bs, :], start=False, stop=True)
                        nc.scalar.activation(hcs[bs, :], hct[bi][0:64, :],
                                             AF.Tanh, bias=bu)
                st[p]["hcs"] = hcs

            def ph_update(p, t):
                zS, hcs = st[p]["zS"], st[p]["hcs"]
                hnew = sbp.tile([128, N], BF, tag=f"hw{p}")
                if h0_zero and t == 0:
                    # h1 = z * hc (since h == 0)
                    nc.vector.tensor_tensor(hnew[:], zS[:], hcs[:], ALU.mult)
                else:
                    u = st[p]["u"]
                    v = sbp.tile([128, N], BF, tag=f"tmp{p}", bufs=2)
                    if t < T - 1:
                        nc.vector.tensor_tensor(v[:], zS[:], hcs[:], ALU.mult)
                        nc.vector.tensor_tensor(hnew[:], u[:], v[:], ALU.add)
                    else:
                        # final step: halves, so the output DMA overlaps
                        for hh in range(2):
                            sl = slice(hh * (N // 2), (hh + 1) * (N // 2))
                            nc.vector.tensor_tensor(v[:, sl], zS[:, sl],
                                                    hcs[:, sl], ALU.mult)
                            nc.vector.tensor_tensor(hnew[:, sl], u[:, sl],
                                                    v[:, sl], ALU.add)
                            nc.sync.dma_start(HOUT_d[p, :, sl],
                                              hnew[:, sl])
                HW[p] = hnew
                if t < T - 1:
                    HN_s[p] = transp(p, hnew, "hn2", fp8=True)

            def dump(nm, ap):
                if debug:
                    nc.sync.dma_start(dbg[nm][0:ap.shape[0]], ap)

            def ph_dbg(p, t):
                import os
                if not debug or p != 0 or t != int(os.environ.get("DBG_T", "0")):
                    return
                dump("DXA", XA24[:])
                dump("DT01", st[0]["t01g"][0][0][:])
                dump("DAX", st[0]["t01g"][1][0][:])
                dump("DZS", st[0]["zS"][:])
                dump("DRS", st[0]["rS"][:])
                dump("DCH", st[0]["ch_dbg"][:])
                dump("DHCS", st[0]["hcs"][:])
                dump("DHW1", HW[0][:])
                dump("DHN", HN_s[0][:])
                dump("DCN", st[0]["CN"][:])

            PHASES = [ph_agg_gate, ph_gate_w, ph_rt, ph_agg_cand,
                      ph_upd_w, ph_update, ph_dbg]
            NPH = len(PHASES)
            OFF = OFF_TICKS
            xa_precompute()
            for tick in range(NPH * T + OFF):
                for p in range(2):
                    local = tick - OFF * p
                    if 0 <= local < NPH * T:
                        t, ph = divmod(local, NPH)
                        PHASES[ph](p, t)

    _split_excess_waits(nc, max_waits=1)
    return nc


_NC_CACHE = {}


def _get_nc(debug=False, h0_zero=False):
    key = f"nc{debug}{h0_zero}{CAND_FP8}{OFF_TICKS}"
    if key not in _NC_CACHE:
        _NC_CACHE[key] = _build_nc(debug, h0_zero)
    return _NC_CACHE[key]


def _host_prep(G, x_seq, init_h, W_gate, b_gate, W_update, b_update):
    f32 = np.float32
    GTf = np.asarray(G, np.float32).transpose(0, 2, 1)         # [k, jn, i]
    GT = GTf.reshape(K, NT, 128, N).transpose(1, 2, 0, 3)      # [j, p, k, i]
    GT = np.ascontiguousarray(GT).reshape(NT, 128, K * N).astype(NPBF)
    # GT8[p, jp, k, kt, i] = fp8(64 * G_k[i, (2jp+kt)*128+p])
    G8 = (64.0 * GTf).reshape(K, NT // 2, 2, 128, N)           # [k, jp, kt, p, i]
    G8 = G8.transpose(3, 1, 0, 2, 4)                           # [p, jp, k, kt, i]
    GT8 = np.ascontiguousarray(G8).reshape(
        128, (NT // 2) * K * 2 * N).astype(NPF8)
    WG3 = np.asarray(W_gate, f32).reshape(K, P, 2 * H)
    WU3 = np.asarray(W_update, f32).reshape(K, P, H)
    WG0 = np.concatenate([WG3[0, C:, :]] * 2, axis=0)
    WG1 = np.concatenate([WG3[1, C:, :]] * 2, axis=0)
    WU0 = np.concatenate([WU3[0, C:, :]] * 2, axis=0)
    WU1 = np.concatenate([WU3[1, C:, :]] * 2, axis=0)
    # x-block rows (k,c): row k*2+c = W[k, c, :]
    xg = WG3[:, :C, :].reshape(K * C, 2 * H)
    xu = WU3[:, :C, :].reshape(K * C, H)
    WGX2 = np.concatenate([WG3[2, C:, :], xg], axis=0)
    WUX2 = np.concatenate([WU3[2, C:, :], xu], axis=0)
    WB = np.zeros((128, 706), f32)
    WB[:, 0:128] = WG0 / 1024.0
    WB[:, 128:256] = WG1 / 1024.0
    WB[0:70, 256:384] = WGX2
    WB[0:64, 256:384] = WGX2[0:64] / 1024.0
    usc = 1024.0 if CAND_FP8 else 1.0
    WB[:, 384:448] = WU0 / usc
    WB[:, 448:512] = WU1 / usc
    WB[0:70, 512:576] = WUX2
    WB[0:64, 512:576] = WUX2[0:64] / usc
    WB[:, 576:704] = np.eye(128, dtype=f32)
    WB[:, 704] = np.asarray(b_gate, f32)
    WB[0:64, 705] = np.asarray(b_update, f32)
    shared = {
        "GT": GT,
        "WB": WB.astype(NPBF),
        "GT8": GT8,
    }
    x_seq = np.asarray(x_seq, f32)
    init_h = np.asarray(init_h, f32)
    in_maps = []
    for c in range(NCORES):
        b0 = c * BL
        xs = x_seq[b0:b0 + BL]                     # [4, 12, 512, 2]
        h0 = init_h[b0:b0 + BL]                    # [4, 512, 64]
        m = dict(shared)
        # XN cols (b, c, t)
        m["XN"] = np.ascontiguousarray(
            xs.transpose(2, 0, 3, 1)).reshape(N, BL * T * C).astype(NPBF)
        # HN0[p][n_loc, j*128 + b*64 + f] = h0[2p+b, j*128+n_loc, f]
        hn = h0.reshape(2, 2, NT, 128, H)          # [p, b, j, n, f]
        m["HN0"] = (16.0 * np.ascontiguousarray(
            hn.transpose(0, 3, 2, 1, 4)).reshape(2, 128, N)).astype(NPF8)
        # HW0[p][b*64 + f, i] = h0[2p+b, i, f]
        hw = h0.reshape(2, 2, N, H)                # [p, b, i, f]
        m["HW0"] = np.ascontiguousarray(
            hw.transpose(0, 1, 3, 2)).reshape(2, 128, N).astype(NPBF)
        in_maps.append(m)
    return in_maps


def _run(inputs, trace=False, debug=False):
    h0_zero = not np.any(np.asarray(inputs["init_h"]))
    nc = _get_nc(debug, h0_zero)
    in_maps = _host_prep(**inputs)
    res = run_bass_kernel_spmd(nc, in_maps, list(range(NCORES)), trace=trace)
    outs = []
    for c in range(NCORES):
        hout = np.asarray(res.results[c]["HOUT"], dtype=np.float32)
        # [2, 128, 512] -> [4, 512, 64]
        hout = hout.reshape(2, 2, H, N).transpose(0, 1, 3, 2).reshape(
            BL, N, H)
        outs.append(hout)
    full = np.concatenate(outs, axis=0).astype(np.float32)
    return full, res


def kernel(G, x_seq, init_h, W_gate, b_gate, W_update, b_update):
    full, _ = _run(dict(G=G, x_seq=x_seq, init_h=init_h, W_gate=W_gate,
                        b_gate=b_gate, W_update=W_update, b_update=b_update))
    return full


# revision 27
# speedup vs baseline: 1.1586x; 1.1149x over previous
"""GCN-GRU encoder (DCRNN-style) on 8 TRN2 NeuronCores, data-parallel over B.

v3: batch-stacked layout. Per core (B_loc=4 = 2 pairs):
  HW[p]   [128, 512] bf16  GRU state, row = bi*64 + f, col = node i
  HN_s[p] [128, 512] fp8   node-major 16*h, col = j*128 + bi*64 + f (agg lhsT)
  CN_s[p] same for r*h (bf16, or fp8 when CAND_FP8)
  gt[k][j] [128, 512] bf16 G[k].T j-tile (agg rhs)
  tAB[k]  [128, 512] bf16  per-pair hop-k aggregates, row = bi*64 + f
  ax[bi]  [70, 512]        k2 aggregate rows 0:64 + 6 static x-aggregate rows
Gates: per-batch matmul -> zrt [128 = z|r, 512] PSUM; 2 sigmoids per batch
write z into zS[bi*64:+64] and r into rS (batch-stacked [128,512] tiles).
GRU elementwise entirely in [128, 512] bf16 (full 128 DVE lanes).
Transposes: 4x 128x128 PE transposes per [128,512] tile.
"""
import numpy as np
import ml_dtypes

import concourse.bass as bass
import concourse.tile as tile
from concourse import mybir
from concourse.bass_utils import run_bass_kernel_spmd

dt = mybir.dt
AF = mybir.ActivationFunctionType
ALU = mybir.AluOpType

B, T, N, C, H, K = 32, 12, 512, 2, 64, 3
NCORES = 8
BL = B // NCORES          # 4 batches per core
NT = N // 128             # 4 partition tiles of the node dim
P = C + H                 # 66
BF = dt.bfloat16
NPBF = ml_dtypes.bfloat16
NPF8 = ml_dtypes.float8_e4m3fn

_waitsplit_ctr = [0]

# tuning knobs (module-level so experiments can flip them before build)
CAND_FP8 = False      # candidate aggregation in fp8 DoubleRow
OFF_TICKS = 1         # phase offset between the two batch pairs
K2X = False           # k2 aggregate as shared [128,512] tile + direct-XA24
                      # x matmuls (drops ax tiles / xcopy / memset)
GATE_DR = False       # gate k0k1 weight matmul in fp8 DoubleRow (needs K2X)
UPD_DR = False        # update k0k1 weight matmul in fp8 DoubleRow (needs K2X)
SCL = 64.0            # pre-activation scale carried in PSUM (act scale=1/SCL)


def _split_excess_waits(nc, max_waits=1):
    """This walrus build allows only `max_waits` semaphore waits per
    instruction; hoist the excess onto preceding same-engine NoOps."""
    for f in nc.m.functions:
        for blk in f.blocks:
            new = []
            for inst in blk.instructions:
                si = inst.sync_info
                if si is not None and len(si.on_wait) > max_waits:
                    waits = list(si.on_wait)
                    head, tail = waits[:-max_waits], waits[-max_waits:]
                    for s in range(0, len(head), max_waits):
                        nop = mybir.InstNoOp(
                            name=f"I-waitsplit-{_waitsplit_ctr[0]}", ins=[], outs=[])
                        _waitsplit_ctr[0] += 1
                        nop.engine = inst.engine
                        nop.sync_info = mybir.SyncInfo(
                            on_wait=list(head[s:s + max_waits]), on_update=[])
                        new.append(nop)
                    inst.sync_info = mybir.SyncInfo(
                        on_wait=list(tail), on_update=list(si.on_update))
                new.append(inst)
            blk.instructions[:] = new


def _build_nc(debug=False, h0_zero=False):
    nc = bass.Bass()
    f32 = dt.float32
    GT_d = nc.declare_dram_parameter("GT", [NT, 128, K * N], BF,
                                     isOutput=False)
    XN_d = nc.declare_dram_parameter("XN", [N, BL * T * C], BF, isOutput=False)
    HN0_d = nc.declare_dram_parameter("HN0", [2, 128, N],
                                      dt.float8e4, isOutput=False)
    HW0_d = nc.declare_dram_parameter("HW0", [2, 128, N], BF, isOutput=False)
    # weight blob cols: wg0 0:128 | wg1 128:256 | wg2/wgx2 256:384 |
    # wu0 384:448 | wu1 448:512 | wu2/wux2 512:576 | eye128 576:704 |
    # bg 704:705 | bu 705:706 | wxg 706:834 | wxu 834:898   (biases bf16)
    WB_d = nc.declare_dram_parameter("WB", [128, 898], BF, isOutput=False)
    # fp8 weight blob (DoubleRow weight matmuls): wg01 [128,2,128] cols
    # 0:256 | wu01 [128,2,64] cols 256:384
    WB8_d = nc.declare_dram_parameter("WB8", [128, 384], dt.float8e4,
                                      isOutput=False)
    F8 = dt.float8e4
    GT8_d = nc.declare_dram_parameter("GT8", [128, (NT // 2) * K * 2 * N], F8,
                                      isOutput=False)
    HOUT_d = nc.declare_dram_parameter("HOUT", [2, 128, N], BF, isOutput=True)
    dbg = {}
    if debug:
        for nm, shp in [("DXA", [102, T * N]), ("DT01", [128, N]),
                        ("DAX", [70, N]), ("DZS", [128, N]),
                        ("DRS", [128, N]), ("DCH", [128, N]),
                        ("DHCS", [128, N]), ("DHW1", [128, N]),
                        ("DHN", [128, N]), ("DCN", [128, N])]:
            dbg[nm] = nc.declare_dram_parameter(nm, shp, BF, isOutput=True)
    XAS_d = nc.dram_tensor("XAS_scratch", [K, BL * T * C, N], BF)

    with tile.TileContext(nc) as tc:
        with tc.tile_pool(name="const", bufs=1) as cst, \
             tc.tile_pool(name="t01s", bufs=2) as t01p, \
             tc.tile_pool(name="sb", bufs=2) as sbp, \
             tc.tile_pool(name="aggps", bufs=4, space="PSUM") as aggps, \
             tc.tile_pool(name="zrps", bufs=2, space="PSUM") as zrps, \
             tc.tile_pool(name="trps", bufs=2, space="PSUM") as trps:

            def load(shape, src_ap, tag, dtype=BF):
                d = cst.tile(shape, dtype, tag=tag)
                nc.sync.dma_start(d[:], src_ap)
                return d

            # ---- constants / inputs (gt halves around xn so the XA
            # precompute can start as early as possible) ----
            gtall = cst.tile([128, NT * K * N], BF, tag="gtall")
            half = NT * K * N // 2
            nc.sync.dma_start(
                gtall[:, 0:half].rearrange("p (j ki) -> p j ki", j=NT // 2),
                GT_d[0:NT // 2].rearrange("j p ki -> p j ki"))
            xnall = cst.tile([128, NT * BL * T * C], BF, tag="xnall")
            nc.sync.dma_start(
                xnall[:].rearrange("p (j q) -> p j q", j=NT),
                XN_d.rearrange("(j p) q -> p j q", j=NT))
            xn = [xnall[:, j * BL * T * C:(j + 1) * BL * T * C]
                  for j in range(NT)]
            nc.sync.dma_start(
                gtall[:, half:].rearrange("p (j ki) -> p j ki", j=NT // 2),
                GT_d[NT // 2:].rearrange("j p ki -> p j ki"))
            gt = [[gtall[:, (j * K + k) * N:(j * K + k + 1) * N]
                   for j in range(NT)] for k in range(K)]
            gt8all = cst.tile([128, (NT // 2) * K * 2 * N], F8, tag="gt8all")
            nc.sync.dma_start(gt8all[:], GT8_d[:])
            # gt8[jp][k]: [128, 2, N] fp8 double-row operand
            gt8 = [[gt8all[:, ((jp * K + k) * 2) * N:
                           ((jp * K + k) * 2 + 2) * N].rearrange(
                               "p (kt i) -> p kt i", kt=2)
                    for k in range(K)] for jp in range(NT // 2)]

            HN_s = [None, None]
            HW = [None, None]
            if not h0_zero:
                for p in range(2):
                    hn0 = sbp.tile([128, N], F8, tag=f"hn{p}",
                                   name=f"hn0{p}")
                    nc.sync.dma_start(hn0[:], HN0_d[p])
                    HN_s[p] = hn0
                    hw0 = sbp.tile([128, N], BF, tag=f"hw{p}",
                                   name=f"hw0{p}")
                    nc.sync.dma_start(hw0[:], HW0_d[p])
                    HW[p] = hw0
            wb = load([128, 898], WB_d[:], "wb")
            wg0 = wb[:, 0:128]
            wg1 = wb[:, 128:256]
            wgx2 = wb[0:70, 256:384]
            wg2 = wb[:, 256:384]
            wu0 = wb[:, 384:448]
            wu1 = wb[:, 448:512]
            wux2 = wb[0:70, 512:576]
            wu2 = wb[:, 512:576]
            eye = wb[:, 576:704]
            bg = wb[:, 704:705]
            bu = wb[0:64, 705:706]
            wxg = wb[:, 706:834]
            wxu = wb[:, 834:898]
            if GATE_DR or UPD_DR:
                wb8 = load([128, 384], WB8_d[:], "wb8", dtype=F8)
                wg01 = wb8[:, 0:256].rearrange("p (kt m) -> p kt m", kt=2)
                wu01 = wb8[:, 256:384].rearrange("p (kt m) -> p kt m", kt=2)
            S_DG = SCL / 1024.0
            # update drain scale: psum carries 1024*agg under CAND_FP8,
            # plain agg otherwise; wu01_8 carries the rest of SCL
            S_DU = (SCL / 1024.0) if CAND_FP8 else 16.0

            # static x-aggregates: row b*32 + k*2 + c, col t*512+i
            XA24 = cst.tile([(BL - 1) * 32 + C * K, T * N], BF, tag="xa24")


            # ---- x aggregation precompute (emitted after the first
            # h-aggregations so PE starts on loop work immediately) ----
            def xa_precompute():
                for k in range(K):
                    ps = aggps.tile([BL * T * C, N], f32, tag="agg")
                    for j in range(NT):
                        nc.tensor.matmul(ps[:], xn[j], gt[k][j],
                                         start=(j == 0), stop=(j == NT - 1))
                    xas = sbp.tile([BL * T * C, N], BF, tag=f"xas{k}",
                                   name=f"xas{k}")
                    nc.vector.tensor_copy(xas[:], ps[:])
                    for b in range(BL):
                        eng = nc.sync if b < 3 else nc.gpsimd
                        for c in range(C):
                            row = b * 32 + k * 2 + c
                            eng.dma_start(
                                XA24[row:row + 1, :],
                                xas[b * 24 + c * T:b * 24 + (c + 1) * T, :])

            # ---- per-step phase bodies ----
            # k2 drains rotate DVE / Act
            def drain(ci, dst_ap, src_ap):
                eng = (nc.vector.tensor_copy, nc.scalar.copy)[ci % 2]
                i_ = eng(dst_ap, src_ap)
                i_.ins.bass_priority = -20

            st = [dict(), dict()]

            def xcopy(p, t, ax2):
                for bi in range(2):
                    b = 2 * p + bi
                    i_ = nc.vector.tensor_copy(ax2[bi][64:70, :],
                                               XA24[b * 32:b * 32 + 6,
                                                    t * N:(t + 1) * N])
                    i_.ins.bass_priority = -20

            def agg_zero(p, t, dtag):
                """t=0 with h==0: aggregates are all zero; with K2X the
                weight phase reads XA24 directly, nothing to build."""
                if K2X:
                    return None
                ax2 = []
                for bi in range(2):
                    ax = t01p.tile([70, N], BF, tag=f"ax{dtag}{p}{bi}",
                                   name=f"axz{dtag}{p}{bi}")
                    nc.vector.memset(ax[0:64, :], 0.0)
                    ax2.append(ax)
                xcopy(p, t, ax2)
                return dict(ax2=ax2, zero=True)

            def agg(p, t, src, dtag, fp8=False, dr=False, s_d=1.0):
                """3-hop aggregation of node-major src.
                k0/k1 -> pair tiles (bf16 tAB, or one fp8 [128,2,N] t01
                when dr). k2 -> shared k2S tile (K2X) or per-batch [70,N]
                ax tiles carrying the 6 static x rows (legacy).
                fp8: double-row agg matmuls (src is an fp8 node-major)."""
                res = dict(zero=False)
                if not K2X:
                    ax2 = []
                    for bi in range(2):
                        ax = t01p.tile([70, N], BF, tag=f"ax{dtag}{p}{bi}",
                                       name=f"ax{dtag}{p}{bi}")
                        ax2.append(ax)
                    xcopy(p, t, ax2)
                    res["ax2"] = ax2
                psk = {}
                for k in (2, 0, 1):
                    ps = aggps.tile([128, N], f32, tag="agg")
                    if fp8:
                        for jp in range(NT // 2):
                            lhs = src[:, jp * 256:(jp + 1) * 256].rearrange(
                                "p (kt m) -> p kt m", kt=2)
                            nc.tensor.matmul(
                                ps[:], lhs, gt8[jp][k],
                                start=(jp == 0), stop=(jp == NT // 2 - 1),
                                perf_mode=mybir.MatmulPerfMode.DoubleRow)
                    else:
                        for j in range(NT):
                            nc.tensor.matmul(
                                ps[:], src[:, j * 128:(j + 1) * 128],
                                gt[k][j],
                                start=(j == 0), stop=(j == NT - 1))
                    psk[k] = ps
                    if k == 2:
                        if K2X:
                            k2S = t01p.tile([128, N], BF, tag=f"k2{dtag}{p}")
                            drain(p, k2S[:], ps[:])
                            res["k2S"] = k2S
                        else:
                            for bi in range(2):
                                drain(2 * p + bi, ax2[bi][0:64, :],
                                      ps[bi * 64:bi * 64 + 64, :])
                if dr:
                    t01 = t01p.tile([128, 2, N], F8, tag=f"{dtag}{p}8")
                    i_ = nc.vector.tensor_scalar_mul(t01[:, 0, :],
                                                     psk[0][:], s_d)
                    i_.ins.bass_priority = -20
                    i_ = nc.scalar.activation(t01[:, 1, :], psk[1][:],
                                              AF.Copy, scale=s_d)
                    i_.ins.bass_priority = -20
                    res["t01"] = t01
                else:
                    tAB = []
                    for k in range(2):
                        tt = t01p.tile([128, N], BF, tag=f"{dtag}{p}{k}")
                        # spread the two big hop drains: k0 -> DVE, k1 -> Act
                        if k == 0:
                            i_ = nc.vector.tensor_copy(tt[:], psk[k][:])
                        else:
                            i_ = nc.scalar.copy(tt[:], psk[k][:])
                        i_.ins.bass_priority = -20
                        tAB.append(tt)
                    res["tAB"] = tAB
                return res

            def transp(p, src, dst_tag, fp8=False):
                """batch-stacked [128, 512] bf16 -> node-major [128, 512];
                drained per half so the next agg's first matmul never
                waits the full transpose set."""
                trp = trps.tile([128, N], BF, tag="tr")
                d = sbp.tile([128, N], F8 if fp8 else BF,
                             tag=f"{dst_tag}{p}", name=f"{dst_tag}{p}")
                for j in range(NT):
                    nc.tensor.transpose(
                        trp[:, j * 128:(j + 1) * 128],
                        src[:, j * 128:(j + 1) * 128],
                        eye)
                for hh in range(2):
                    sl = slice(hh * (N // 2), (hh + 1) * (N // 2))
                    if fp8:
                        i_ = nc.vector.tensor_scalar_mul(d[:, sl], trp[:, sl],
                                                         16.0)
                    else:
                        i_ = nc.vector.tensor_copy(d[:, sl], trp[:, sl])
                    i_.ins.bass_priority = -20
                return d

            def ph_agg_gate(p, t):
                if h0_zero and t == 0:
                    st[p]["t01g"] = agg_zero(p, t, "tg")
                else:
                    st[p]["t01g"] = agg(p, t, HN_s[p], "tg",
                                        fp8=True, dr=GATE_DR, s_d=S_DG)

            def ph_gate_w(p, t):
                res = st[p]["t01g"]
                zS = sbp.tile([128, N], BF, tag=f"zs{p}")
                rS = sbp.tile([128, N], BF, tag=f"rs{p}")
                zrt = [zrps.tile([128, N], f32, tag="zr", name=f"zr{p}{bi}")
                       for bi in range(2)]
                zero = res is None or res.get("zero")
                for bi in range(2):
                    bs = slice(bi * 64, bi * 64 + 64)
                    b = 2 * p + bi
                    xsl = slice(b * 32, b * 32 + 6)
                    if K2X:
                        nc.tensor.matmul(zrt[bi][:], wxg[xsl, :],
                                         XA24[xsl, t * N:(t + 1) * N],
                                         start=True, stop=zero)
                        if not zero:
                            nc.tensor.matmul(zrt[bi][:], wg2[bs, :],
                                             res["k2S"][bs, :],
                                             start=False, stop=False)
                    else:
                        ax2 = res["ax2"]
                        nc.tensor.matmul(zrt[bi][:], wgx2, ax2[bi][:],
                                         start=True, stop=zero)
                    if not zero and not GATE_DR:
                        nc.tensor.matmul(zrt[bi][:], wg0[bs, :],
                                         res["tAB"][0][bs, :],
                                         start=False, stop=False)
                for bi in range(2):
                    bs = slice(bi * 64, bi * 64 + 64)
                    if not zero:
                        if GATE_DR:
                            nc.tensor.matmul(
                                zrt[bi][:], wg01[bs, :, :],
                                res["t01"][bs, :, :],
                                start=False, stop=True,
                                perf_mode=mybir.MatmulPerfMode.DoubleRow)
                        else:
                            nc.tensor.matmul(zrt[bi][:], wg1[bs, :],
                                             res["tAB"][1][bs, :],
                                             start=False, stop=True)
                    # r first: the r*h product is the critical path
                    nc.scalar.activation(rS[bs, :], zrt[bi][64:128, :],
                                         AF.Sigmoid, bias=bg[64:128, :],
                                         scale=1.0 / SCL)
                for bi in range(2):
                    bs = slice(bi * 64, bi * 64 + 64)
                    nc.scalar.activation(zS[bs, :], zrt[bi][0:64, :],
                                         AF.Sigmoid, bias=bg[0:64, :],
                                         scale=1.0 / SCL)
                st[p]["zS"], st[p]["rS"] = zS, rS

            def ph_rt(p, t):
                if h0_zero and t == 0:
                    return
                zS, rS = st[p]["zS"], st[p]["rS"]
                ch = sbp.tile([128, N], BF, tag=f"ch{p}")
                nc.vector.tensor_tensor(ch[:], rS[:], HW[p][:], ALU.mult)
                st[p]["CN"] = transp(p, ch, "cn", fp8=CAND_FP8)
                st[p]["ch_dbg"] = ch
                # u = h - z*h = (1-z)*h, off the critical path
                zh = sbp.tile([128, N], BF, tag=f"tmp{p}", bufs=2,
                              name=f"zh{p}")
                nc.gpsimd.tensor_tensor(zh[:], zS[:], HW[p][:], ALU.mult)
                u = sbp.tile([128, N], BF, tag=f"u{p}")
                nc.gpsimd.tensor_tensor(u[:], HW[p][:], zh[:], ALU.subtract)
                st[p]["u"] = u

            def ph_agg_cand(p, t):
                if h0_zero and t == 0:
                    st[p]["t01u"] = agg_zero(p, t, "tu")
                else:
                    st[p]["t01u"] = agg(p, t, st[p]["CN"], "tu",
                                        fp8=CAND_FP8, dr=UPD_DR, s_d=S_DU)

            def ph_upd_w(p, t):
                res = st[p]["t01u"]
                hcs = sbp.tile([128, N], BF, tag=f"hcs{p}")
                hct = [zrps.tile([128, N], f32, tag="zr", name=f"hc{p}{bi}")
                       for bi in range(2)]
                zero = res is None or res.get("zero")
                for bi in range(2):
                    bs = slice(bi * 64, bi * 64 + 64)
                    b = 2 * p + bi
                    xsl = slice(b * 32, b * 32 + 6)
                    if K2X:
                        nc.tensor.matmul(hct[bi][0:64, :], wxu[xsl, :],
                                         XA24[xsl, t * N:(t + 1) * N],
                                         start=True, stop=zero)
                        if not zero:
                            nc.tensor.matmul(hct[bi][0:64, :], wu2[bs, :],
                                             res["k2S"][bs, :],
                                             start=False, stop=False)
                    else:
                        nc.tensor.matmul(hct[bi][0:64, :], wux2,
                                         res["ax2"][bi][:],
                                         start=True, stop=zero)
                    if not zero and not UPD_DR:
                        nc.tensor.matmul(hct[bi][0:64, :], wu0[bs, :],
                                         res["tAB"][0][bs, :], start=False,
                                         stop=False)
                for bi in range(2):
                    bs = slice(bi * 64, bi * 64 + 64)
                    if not zero:
                        if UPD_DR:
                            nc.tensor.matmul(
                                hct[bi][0:64, :], wu01[bs, :, :],
                                res["t01"][bs, :, :],
                                start=False, stop=True,
                                perf_mode=mybir.MatmulPerfMode.DoubleRow)
                        else:
                            nc.tensor.matmul(hct[bi][0:64, :], wu1[bs, :],
                                             res["tAB"][1][bs, :],
                                             start=False, stop=True)
                    nc.scalar.activation(hcs[bs, :], hct[bi][0:64, :],
                                         AF.Tanh, bias=bu, scale=1.0 / SCL)
                st[p]["hcs"] = hcs

            def ph_update(p, t):
                zS, hcs = st[p]["zS"], st[p]["hcs"]
                hnew = sbp.tile([128, N], BF, tag=f"hw{p}")
                if h0_zero and t == 0:
                    # h1 = z * hc (since h == 0)
                    nc.vector.tensor_tensor(hnew[:], zS[:], hcs[:], ALU.mult)
                else:
                    u = st[p]["u"]
                    v = sbp.tile([128, N], BF, tag=f"tmp{p}", bufs=2)
                    if t < T - 1:
                        nc.vector.tensor_tensor(v[:], zS[:], hcs[:], ALU.mult)
                        nc.vector.tensor_tensor(hnew[:], u[:], v[:], ALU.add)
                    else:
                        # final step: halves, so the output DMA overlaps
                        for hh in range(2):
                            sl = slice(hh * (N // 2), (hh + 1) * (N // 2))
                            nc.vector.tensor_tensor(v[:, sl], zS[:, sl],
                                                    hcs[:, sl], ALU.mult)
                            nc.vector.tensor_tensor(hnew[:, sl], u[:, sl],
                                                    v[:, sl], ALU.add)
                            nc.sync.dma_start(HOUT_d[p, :, sl],
                                              hnew[:, sl])
                HW[p] = hnew
                if t < T - 1:
                    HN_s[p] = transp(p, hnew, "hn2", fp8=True)

            def dump(nm, ap):
                if debug:
                    nc.sync.dma_start(dbg[nm][0:ap.shape[0]], ap)

            def ph_dbg(p, t):
                import os
                if not debug or p != 0 or t != int(os.environ.get("DBG_T", "0")):
                    return
                dump("DXA", XA24[:])
                dump("DZS", st[0]["zS"][:])
                dump("DRS", st[0]["rS"][:])
                dump("DCH", st[0]["ch_dbg"][:])
                dump("DHCS", st[0]["hcs"][:])
                dump("DHW1", HW[0][:])
                dump("DHN", HN_s[0][:])
                dump("DCN", st[0]["CN"][:])

            PHASES = [ph_agg_gate, ph_gate_w, ph_rt, ph_agg_cand,
                      ph_upd_w, ph_update, ph_dbg]
            NPH = len(PHASES)
            OFF = OFF_TICKS
            xa_precompute()
            for tick in range(NPH * T + OFF):
                for p in range(2):
                    local = tick - OFF * p
                    if 0 <= local < NPH * T:
                        t, ph = divmod(local, NPH)
                        PHASES[ph](p, t)

    _split_excess_waits(nc, max_waits=1)
    return nc


_NC_CACHE = {}


def _get_nc(debug=False, h0_zero=False):
    key = f"nc{debug}{h0_zero}{CAND_FP8}{OFF_TICKS}{K2X}{GATE_DR}{UPD_DR}"
    if key not in _NC_CACHE:
        _NC_CACHE[key] = _build_nc(debug, h0_zero)
    return _NC_CACHE[key]


def _host_prep(G, x_seq, init_h, W_gate, b_gate, W_update, b_update):
    f32 = np.float32
    GTf = np.asarray(G, np.float32).transpose(0, 2, 1)         # [k, jn, i]
    GT = GTf.reshape(K, NT, 128, N).transpose(1, 2, 0, 3)      # [j, p, k, i]
    GT = np.ascontiguousarray(GT).reshape(NT, 128, K * N).astype(NPBF)
    # GT8[p, jp, k, kt, i] = fp8(64 * G_k[i, (2jp+kt)*128+p])
    G8 = (64.0 * GTf).reshape(K, NT // 2, 2, 128, N)           # [k, jp, kt, p, i]
    G8 = G8.transpose(3, 1, 0, 2, 4)                           # [p, jp, k, kt, i]
    GT8 = np.ascontiguousarray(G8).reshape(
        128, (NT // 2) * K * 2 * N).astype(NPF8)
    WG3 = np.asarray(W_gate, f32).reshape(K, P, 2 * H)
    WU3 = np.asarray(W_update, f32).reshape(K, P, H)
    WG0 = np.concatenate([WG3[0, C:, :]] * 2, axis=0)
    WG1 = np.concatenate([WG3[1, C:, :]] * 2, axis=0)
    WG2 = np.concatenate([WG3[2, C:, :]] * 2, axis=0)
    WU0 = np.concatenate([WU3[0, C:, :]] * 2, axis=0)
    WU1 = np.concatenate([WU3[1, C:, :]] * 2, axis=0)
    WU2 = np.concatenate([WU3[2, C:, :]] * 2, axis=0)
    # x-block rows (k,c): row k*2+c = W[k, c, :]
    xg = WG3[:, :C, :].reshape(K * C, 2 * H)
    xu = WU3[:, :C, :].reshape(K * C, H)
    WGX2 = np.concatenate([WG3[2, C:, :], xg], axis=0)
    WUX2 = np.concatenate([WU3[2, C:, :], xu], axis=0)
    S = SCL
    usc = 1024.0 if CAND_FP8 else 1.0
    WB = np.zeros((128, 898), f32)
    WB[:, 0:128] = S * WG0 / 1024.0
    WB[:, 128:256] = S * WG1 / 1024.0
    if K2X:
        WB[:, 256:384] = S * WG2 / 1024.0
        WB[:, 512:576] = S * WU2 / usc
        for b in range(BL):
            WB[b * 32:b * 32 + 6, 706:834] = S * xg
            WB[b * 32:b * 32 + 6, 834:898] = S * xu
    else:
        WB[0:70, 256:384] = S * WGX2
        WB[0:64, 256:384] = S * WGX2[0:64] / 1024.0
        WB[0:70, 512:576] = S * WUX2
        WB[0:64, 512:576] = S * WUX2[0:64] / usc
    WB[:, 384:448] = S * WU0 / usc
    WB[:, 448:512] = S * WU1 / usc
    WB[:, 576:704] = np.eye(128, dtype=f32)
    WB[:, 704] = S * np.asarray(b_gate, f32)
    WB[0:64, 705] = S * np.asarray(b_update, f32)
    # fp8 DoubleRow weight blob: product scale must come out at S
    # gate: t01g = (1024*agg) * (S/1024) -> w8 = W
    # update: t01u = agg * 16 (or 1024*agg * S/1024) -> w8 = W*4 (or W)
    WB8 = np.zeros((128, 384), f32)
    wgk = np.stack([WG3[0, C:, :], WG3[1, C:, :]], axis=1)    # [64,2,128]
    WB8[:, 0:256] = np.concatenate([wgk] * 2, axis=0).reshape(128, 256)
    s_wu = 1.0 if CAND_FP8 else 4.0
    wuk = np.stack([WU3[0, C:, :], WU3[1, C:, :]], axis=1) * s_wu
    WB8[:, 256:384] = np.concatenate([wuk] * 2, axis=0).reshape(128, 128)
    shared = {
        "GT": GT,
        "WB": WB.astype(NPBF),
        "WB8": WB8.astype(NPF8),
        "GT8": GT8,
    }
    x_seq = np.asarray(x_seq, f32)
    init_h = np.asarray(init_h, f32)
    in_maps = []
    for c in range(NCORES):
        b0 = c * BL
        xs = x_seq[b0:b0 + BL]                     # [4, 12, 512, 2]
        h0 = init_h[b0:b0 + BL]                    # [4, 512, 64]
        m = dict(shared)
        # XN cols (b, c, t)
        m["XN"] = np.ascontiguousarray(
            xs.transpose(2, 0, 3, 1)).reshape(N, BL * T * C).astype(NPBF)
        # HN0[p][n_loc, j*128 + b*64 + f] = h0[2p+b, j*128+n_loc, f]
        hn = h0.reshape(2, 2, NT, 128, H)          # [p, b, j, n, f]
        m["HN0"] = (16.0 * np.ascontiguousarray(
            hn.transpose(0, 3, 2, 1, 4)).reshape(2, 128, N)).astype(NPF8)
        # HW0[p][b*64 + f, i] = h0[2p+b, i, f]
        hw = h0.reshape(2, 2, N, H)                # [p, b, i, f]
        m["HW0"] = np.ascontiguousarray(
            hw.transpose(0, 1, 3, 2)).reshape(2, 128, N).astype(NPBF)
        in_maps.append(m)
    return in_maps


def _run(inputs, trace=False, debug=False):
    h0_zero = not np.any(np.asarray(inputs["init_h"]))
    nc = _get_nc(debug, h0_zero)
    in_maps = _host_prep(**inputs)
    res = run_bass_kernel_spmd(nc, in_maps, list(range(NCORES)), trace=trace)
    outs = []
    for c in range(NCORES):
        hout = np.asarray(res.results[c]["HOUT"], dtype=np.float32)
        # [2, 128, 512] -> [4, 512, 64]
        hout = hout.reshape(2, 2, H, N).transpose(0, 1, 3, 2).reshape(
            BL, N, H)
        outs.append(hout)
    full = np.concatenate(outs, axis=0).astype(np.float32)
    return full, res


def kernel(G, x_seq, init_h, W_gate, b_gate, W_update, b_update):
    full, _ = _run(dict(G=G, x_seq=x_seq, init_h=init_h, W_gate=W_gate,
                        b_gate=b_gate, W_update=W_update, b_update=b_update))
    return full


# revision 39
# speedup vs baseline: 1.2246x; 1.0570x over previous
"""GCN-GRU encoder (DCRNN-style) on 8 TRN2 NeuronCores, data-parallel over B.

v3: batch-stacked layout. Per core (B_loc=4 = 2 pairs):
  HW[p]   [128, 512] bf16  GRU state, row = bi*64 + f, col = node i
  HN_s[p] [128, 512] fp8   node-major 16*h, col = j*128 + bi*64 + f (agg lhsT)
  CN_s[p] same for r*h (bf16, or fp8 when CAND_FP8)
  gt[k][j] [128, 512] bf16 G[k].T j-tile (agg rhs)
  tAB[k]  [128, 512] bf16  per-pair hop-k aggregates, row = bi*64 + f
  ax[bi]  [70, 512]        k2 aggregate rows 0:64 + 6 static x-aggregate rows
Gates: per-batch matmul -> zrt [128 = z|r, 512] PSUM; 2 sigmoids per batch
write z into zS[bi*64:+64] and r into rS (batch-stacked [128,512] tiles).
GRU elementwise entirely in [128, 512] bf16 (full 128 DVE lanes).
Transposes: 4x 128x128 PE transposes per [128,512] tile.
"""
import numpy as np
import ml_dtypes

import concourse.bass as bass
import concourse.tile as tile
from concourse import mybir
from concourse.bass_utils import run_bass_kernel_spmd

dt = mybir.dt
AF = mybir.ActivationFunctionType
ALU = mybir.AluOpType

B, T, N, C, H, K = 32, 12, 512, 2, 64, 3
NCORES = 8
BL = B // NCORES          # 4 batches per core
NT = N // 128             # 4 partition tiles of the node dim
P = C + H                 # 66
BF = dt.bfloat16
NPBF = ml_dtypes.bfloat16
NPF8 = ml_dtypes.float8_e4m3fn

_waitsplit_ctr = [0]

# tuning knobs (module-level so experiments can flip them before build)
CAND_FP8 = False      # candidate aggregation in fp8 DoubleRow
OFF_TICKS = 1         # phase offset between the two batch pairs
K2X = True           # k2 aggregate as shared [128,512] tile + direct-XA24
                      # x matmuls (drops ax tiles / xcopy / memset)
GATE_DR = True       # gate k0k1 weight matmul in fp8 DoubleRow (needs K2X)
UPD_DR = False        # update k0k1 weight matmul in fp8 DoubleRow (needs K2X)
SCL = 64.0            # pre-activation scale carried in PSUM (act scale=1/SCL)


def _split_excess_waits(nc, max_waits=1):
    """This walrus build allows only `max_waits` semaphore waits per
    instruction; hoist the excess onto preceding same-engine NoOps."""
    for f in nc.m.functions:
        for blk in f.blocks:
            new = []
            for inst in blk.instructions:
                si = inst.sync_info
                if si is not None and len(si.on_wait) > max_waits:
                    waits = list(si.on_wait)
                    head, tail = waits[:-max_waits], waits[-max_waits:]
                    for s in range(0, len(head), max_waits):
                        nop = mybir.InstNoOp(
                            name=f"I-waitsplit-{_waitsplit_ctr[0]}", ins=[], outs=[])
                        _waitsplit_ctr[0] += 1
                        nop.engine = inst.engine
                        nop.sync_info = mybir.SyncInfo(
                            on_wait=list(head[s:s + max_waits]), on_update=[])
                        new.append(nop)
                    inst.sync_info = mybir.SyncInfo(
                        on_wait=list(tail), on_update=list(si.on_update))
                new.append(inst)
            blk.instructions[:] = new


def _build_nc(debug=False, h0_zero=False):
    nc = bass.Bass()
    f32 = dt.float32
    GT_d = nc.declare_dram_parameter("GT", [NT, 128, K * N], BF,
                                     isOutput=False)
    XN_d = nc.declare_dram_parameter("XN", [N, BL * T * C], BF, isOutput=False)
    HN0_d = nc.declare_dram_parameter("HN0", [2, 128, N],
                                      dt.float8e4, isOutput=False)
    HW0_d = nc.declare_dram_parameter("HW0", [2, 128, N], BF, isOutput=False)
    # weight blob cols: wg0 0:128 | wg1 128:256 | wg2/wgx2 256:384 |
    # wu0 384:448 | wu1 448:512 | wu2/wux2 512:576 | eye128 576:704 |
    # bg 704:705 | bu 705:706 | wxg 706:834 | wxu 834:898   (biases bf16)
    WB_d = nc.declare_dram_parameter("WB", [128, 898], BF, isOutput=False)
    # fp8 weight blob (DoubleRow weight matmuls): wg01 [128,2,128] cols
    # 0:256 | wu01 [128,2,64] cols 256:384
    WB8_d = nc.declare_dram_parameter("WB8", [128, 384], dt.float8e4,
                                      isOutput=False)
    F8 = dt.float8e4
    GT8_d = nc.declare_dram_parameter("GT8", [128, (NT // 2) * K * 2 * N], F8,
                                      isOutput=False)
    HOUT_d = nc.declare_dram_parameter("HOUT", [2, 128, N], BF, isOutput=True)
    dbg = {}
    if debug:
        for nm, shp in [("DXA", [102, T * N]), ("DT01", [128, N]),
                        ("DAX", [70, N]), ("DZS", [128, N]),
                        ("DRS", [128, N]), ("DCH", [128, N]),
                        ("DHCS", [128, N]), ("DHW1", [128, N]),
                        ("DHN", [128, N]), ("DCN", [128, N])]:
            dbg[nm] = nc.declare_dram_parameter(nm, shp, BF, isOutput=True)
    XAS_d = nc.dram_tensor("XAS_scratch", [K, BL * T * C, N], BF)

    with tile.TileContext(nc) as tc:
        with tc.tile_pool(name="const", bufs=1) as cst, \
             tc.tile_pool(name="t01s", bufs=2) as t01p, \
             tc.tile_pool(name="sb", bufs=2) as sbp, \
             tc.tile_pool(name="aggps", bufs=4, space="PSUM") as aggps, \
             tc.tile_pool(name="zrps", bufs=2, space="PSUM") as zrps, \
             tc.tile_pool(name="trps", bufs=2, space="PSUM") as trps:

            def load(shape, src_ap, tag, dtype=BF):
                d = cst.tile(shape, dtype, tag=tag)
                nc.sync.dma_start(d[:], src_ap)
                return d

            # ---- constants / inputs (gt halves around xn so the XA
            # precompute can start as early as possible) ----
            gtall = cst.tile([128, NT * K * N], BF, tag="gtall")
            half = NT * K * N // 2
            nc.sync.dma_start(
                gtall[:, 0:half].rearrange("p (j ki) -> p j ki", j=NT // 2),
                GT_d[0:NT // 2].rearrange("j p ki -> p j ki"))
            xnall = cst.tile([128, NT * BL * T * C], BF, tag="xnall")
            nc.sync.dma_start(
                xnall[:].rearrange("p (j q) -> p j q", j=NT),
                XN_d.rearrange("(j p) q -> p j q", j=NT))
            xn = [xnall[:, j * BL * T * C:(j + 1) * BL * T * C]
                  for j in range(NT)]
            nc.sync.dma_start(
                gtall[:, half:].rearrange("p (j ki) -> p j ki", j=NT // 2),
                GT_d[NT // 2:].rearrange("j p ki -> p j ki"))
            gt = [[gtall[:, (j * K + k) * N:(j * K + k + 1) * N]
                   for j in range(NT)] for k in range(K)]
            gt8all = cst.tile([128, (NT // 2) * K * 2 * N], F8, tag="gt8all")
            nc.sync.dma_start(gt8all[:], GT8_d[:])
            # gt8[jp][k]: [128, 2, N] fp8 double-row operand
            gt8 = [[gt8all[:, ((jp * K + k) * 2) * N:
                           ((jp * K + k) * 2 + 2) * N].rearrange(
                               "p (kt i) -> p kt i", kt=2)
                    for k in range(K)] for jp in range(NT // 2)]

            HN_s = [None, None]
            HW = [None, None]
            if not h0_zero:
                for p in range(2):
                    hn0 = sbp.tile([128, N], F8, tag=f"hn{p}",
                                   name=f"hn0{p}")
                    nc.sync.dma_start(hn0[:], HN0_d[p])
                    HN_s[p] = hn0
                    hw0 = sbp.tile([128, N], BF, tag=f"hw{p}",
                                   name=f"hw0{p}")
                    nc.sync.dma_start(hw0[:], HW0_d[p])
                    HW[p] = hw0
            wb = load([128, 898], WB_d[:], "wb")
            wg0 = wb[:, 0:128]
            wg1 = wb[:, 128:256]
            wgx2 = wb[0:70, 256:384]
            wg2 = wb[:, 256:384]
            wu0 = wb[:, 384:448]
            wu1 = wb[:, 448:512]
            wux2 = wb[0:70, 512:576]
            wu2 = wb[:, 512:576]
            eye = wb[:, 576:704]
            bg = wb[:, 704:705]
            bu = wb[0:64, 705:706]
            bu2 = wb[:, 705:706]
            wxg = wb[:, 706:834]
            wxu = wb[:, 834:898]
            if GATE_DR or UPD_DR:
                wb8 = load([128, 384], WB8_d[:], "wb8", dtype=F8)
                wg01 = wb8[:, 0:256].rearrange("p (kt m) -> p kt m", kt=2)
                wu01 = wb8[:, 256:384].rearrange("p (kt m) -> p kt m", kt=2)
            S_DG = SCL / 1024.0
            # update drain scale: psum carries 1024*agg under CAND_FP8,
            # plain agg otherwise; wu01_8 carries the rest of SCL
            S_DU = (SCL / 1024.0) if CAND_FP8 else 16.0

            # static x-aggregates: row b*32 + k*2 + c, col t*512+i
            # (declared 128 rows so the scatter AP can use uniform strides)
            XA24 = cst.tile([128, T * N], BF, tag="xa24")
            # batch 3 copy at partition base 0 (matmul bases must be 0/32/64)
            XB3 = cst.tile([C * K, T * N], BF, tag="xb3")

            def xa_ap(b, t):
                if b < 3:
                    return (wb[b * 32:b * 32 + 6, 706:834],
                            wb[b * 32:b * 32 + 6, 834:898],
                            XA24[b * 32:b * 32 + 6, t * N:(t + 1) * N])
                return (wb[0:6, 706:834], wb[0:6, 834:898],
                        XB3[0:6, t * N:(t + 1) * N])


            # ---- x aggregation precompute (emitted after the first
            # h-aggregations so PE starts on loop work immediately) ----
            def xa_precompute():
                for k in range(K):
                    ps = aggps.tile([BL * T * C, N], f32, tag="agg")
                    for j in range(NT):
                        nc.tensor.matmul(ps[:], xn[j], gt[k][j],
                                         start=(j == 0), stop=(j == NT - 1))
                    xas = sbp.tile([BL * T * C, N], BF, tag=f"xas{k}",
                                   name=f"xas{k}")
                    nc.vector.tensor_copy(xas[:], ps[:])
                    for b in range(BL):
                        eng = nc.sync if b < 3 else nc.gpsimd
                        for c in range(C):
                            row = b * 32 + k * 2 + c
                            eng.dma_start(
                                XA24[row:row + 1, :],
                                xas[b * 24 + c * T:b * 24 + (c + 1) * T, :])
                            if K2X and b == 3:
                                nc.gpsimd.dma_start(
                                    XB3[k * 2 + c:k * 2 + c + 1, :],
                                    xas[b * 24 + c * T:b * 24 + (c + 1) * T, :])

            # ---- per-step phase bodies ----
            # k2 drains rotate DVE / Act
            def drain(ci, dst_ap, src_ap):
                eng = (nc.vector.tensor_copy, nc.scalar.copy)[ci % 2]
                i_ = eng(dst_ap, src_ap)
                i_.ins.bass_priority = -20

            st = [dict(), dict()]

            def xcopy(p, t, ax2):
                for bi in range(2):
                    b = 2 * p + bi
                    i_ = nc.vector.tensor_copy(ax2[bi][64:70, :],
                                               XA24[b * 32:b * 32 + 6,
                                                    t * N:(t + 1) * N])
                    i_.ins.bass_priority = -20

            def agg_zero(p, t, dtag):
                """t=0 with h==0: aggregates are all zero; with K2X the
                weight phase reads XA24 directly, nothing to build."""
                if K2X:
                    return None
                ax2 = []
                for bi in range(2):
                    ax = t01p.tile([70, N], BF, tag=f"ax{dtag}{p}{bi}",
                                   name=f"axz{dtag}{p}{bi}")
                    nc.vector.memset(ax[0:64, :], 0.0)
                    ax2.append(ax)
                xcopy(p, t, ax2)
                return dict(ax2=ax2, zero=True)

            def agg(p, t, src, dtag, fp8=False, dr=False, s_d=1.0):
                """3-hop aggregation of node-major src.
                k0/k1 -> pair tiles (bf16 tAB, or one fp8 [128,2,N] t01
                when dr). k2 -> shared k2S tile (K2X) or per-batch [70,N]
                ax tiles carrying the 6 static x rows (legacy).
                fp8: double-row agg matmuls (src is an fp8 node-major)."""
                res = dict(zero=False)
                if not K2X:
                    ax2 = []
                    for bi in range(2):
                        ax = t01p.tile([70, N], BF, tag=f"ax{dtag}{p}{bi}",
                                       name=f"ax{dtag}{p}{bi}")
                        ax2.append(ax)
                    xcopy(p, t, ax2)
                    res["ax2"] = ax2
                psk = {}
                for k in (2, 0, 1):
                    ps = aggps.tile([128, N], f32, tag="agg")
                    if fp8:
                        for jp in range(NT // 2):
                            lhs = src[:, jp * 256:(jp + 1) * 256].rearrange(
                                "p (kt m) -> p kt m", kt=2)
                            nc.tensor.matmul(
                                ps[:], lhs, gt8[jp][k],
                                start=(jp == 0), stop=(jp == NT // 2 - 1),
                                perf_mode=mybir.MatmulPerfMode.DoubleRow)
                    else:
                        for j in range(NT):
                            nc.tensor.matmul(
                                ps[:], src[:, j * 128:(j + 1) * 128],
                                gt[k][j],
                                start=(j == 0), stop=(j == NT - 1))
                    psk[k] = ps
                    if k == 2:
                        if K2X:
                            k2S = t01p.tile([128, N], BF, tag=f"k2{dtag}{p}")
                            drain(p, k2S[:], ps[:])
                            res["k2S"] = k2S
                        else:
                            for bi in range(2):
                                drain(2 * p + bi, ax2[bi][0:64, :],
                                      ps[bi * 64:bi * 64 + 64, :])
                if dr:
                    t01 = t01p.tile([128, 2, N], F8, tag=f"{dtag}{p}8")
                    i_ = nc.vector.tensor_scalar_mul(t01[:, 0, :],
                                                     psk[0][:], s_d)
                    i_.ins.bass_priority = -20
                    i_ = nc.scalar.activation(t01[:, 1, :], psk[1][:],
                                              AF.Copy, scale=s_d)
                    i_.ins.bass_priority = -20
                    res["t01"] = t01
                else:
                    tAB = []
                    for k in range(2):
                        tt = t01p.tile([128, N], BF, tag=f"{dtag}{p}{k}")
                        # spread the two big hop drains: k0 -> DVE, k1 -> Act
                        if k == 0:
                            i_ = nc.vector.tensor_copy(tt[:], psk[k][:])
                        else:
                            i_ = nc.scalar.copy(tt[:], psk[k][:])
                        i_.ins.bass_priority = -20
                        tAB.append(tt)
                    res["tAB"] = tAB
                return res

            def transp(p, src, dst_tag, fp8=False):
                """batch-stacked [128, 512] bf16 -> node-major [128, 512];
                drained per half so the next agg's first matmul never
                waits the full transpose set."""
                trp = trps.tile([128, N], BF, tag="tr")
                d = sbp.tile([128, N], F8 if fp8 else BF,
                             tag=f"{dst_tag}{p}", name=f"{dst_tag}{p}")
                for j in range(NT):
                    nc.tensor.transpose(
                        trp[:, j * 128:(j + 1) * 128],
                        src[:, j * 128:(j + 1) * 128],
                        eye)
                for hh in range(2):
                    sl = slice(hh * (N // 2), (hh + 1) * (N // 2))
                    if fp8:
                        i_ = nc.vector.tensor_scalar_mul(d[:, sl], trp[:, sl],
                                                         16.0)
                    else:
                        i_ = nc.vector.tensor_copy(d[:, sl], trp[:, sl])
                    i_.ins.bass_priority = -20
                return d

            def ph_agg_gate(p, t):
                if h0_zero and t == 0:
                    st[p]["t01g"] = agg_zero(p, t, "tg")
                else:
                    st[p]["t01g"] = agg(p, t, HN_s[p], "tg",
                                        fp8=True, dr=GATE_DR, s_d=S_DG)

            def ph_gate_w(p, t):
                res = st[p]["t01g"]
                zS = sbp.tile([128, N], BF, tag=f"zs{p}")
                rS = sbp.tile([128, N], BF, tag=f"rs{p}")
                zrt = [zrps.tile([128, N], f32, tag="zr", name=f"zr{p}{bi}")
                       for bi in range(2)]
                zero = res is None or res.get("zero")
                for bi in range(2):
                    bs = slice(bi * 64, bi * 64 + 64)
                    b = 2 * p + bi
                    if K2X:
                        wxg_ap, _, x_ap = xa_ap(b, t)
                        nc.tensor.matmul(zrt[bi][:], wxg_ap, x_ap,
                                         start=True, stop=zero)
                        if not zero:
                            nc.tensor.matmul(zrt[bi][:], wg2[bs, :],
                                             res["k2S"][bs, :],
                                             start=False, stop=False)
                    else:
                        ax2 = res["ax2"]
                        nc.tensor.matmul(zrt[bi][:], wgx2, ax2[bi][:],
                                         start=True, stop=zero)
                    if not zero and not GATE_DR:
                        nc.tensor.matmul(zrt[bi][:], wg0[bs, :],
                                         res["tAB"][0][bs, :],
                                         start=False, stop=False)
                for bi in range(2):
                    bs = slice(bi * 64, bi * 64 + 64)
                    if not zero:
                        if GATE_DR:
                            nc.tensor.matmul(
                                zrt[bi][:], wg01[bs, :, :],
                                res["t01"][bs, :, :],
                                start=False, stop=True,
                                perf_mode=mybir.MatmulPerfMode.DoubleRow)
                        else:
                            nc.tensor.matmul(zrt[bi][:], wg1[bs, :],
                                             res["tAB"][1][bs, :],
                                             start=False, stop=True)
                    # r first: the r*h product is the critical path
                    nc.scalar.activation(rS[bs, :], zrt[bi][64:128, :],
                                         AF.Sigmoid, bias=bg[64:128, :],
                                         scale=1.0 / SCL)
                for bi in range(2):
                    bs = slice(bi * 64, bi * 64 + 64)
                    nc.scalar.activation(zS[bs, :], zrt[bi][0:64, :],
                                         AF.Sigmoid, bias=bg[0:64, :],
                                         scale=1.0 / SCL)
                st[p]["zS"], st[p]["rS"] = zS, rS

            def ph_rt(p, t):
                if h0_zero and t == 0:
                    return
                zS, rS = st[p]["zS"], st[p]["rS"]
                ch = sbp.tile([128, N], BF, tag=f"ch{p}")
                nc.vector.tensor_tensor(ch[:], rS[:], HW[p][:], ALU.mult)
                st[p]["CN"] = transp(p, ch, "cn", fp8=CAND_FP8)
                st[p]["ch_dbg"] = ch
                # u = h - z*h = (1-z)*h, off the critical path
                zh = sbp.tile([128, N], BF, tag=f"tmp{p}", bufs=2,
                              name=f"zh{p}")
                nc.gpsimd.tensor_tensor(zh[:], zS[:], HW[p][:], ALU.mult)
                u = sbp.tile([128, N], BF, tag=f"u{p}")
                nc.gpsimd.tensor_tensor(u[:], HW[p][:], zh[:], ALU.subtract)
                st[p]["u"] = u

            def ph_agg_cand(p, t):
                if h0_zero and t == 0:
                    st[p]["t01u"] = agg_zero(p, t, "tu")
                else:
                    st[p]["t01u"] = agg(p, t, st[p]["CN"], "tu",
                                        fp8=CAND_FP8, dr=UPD_DR, s_d=S_DU)

            def ph_upd_w(p, t):
                res = st[p]["t01u"]
                hcs = sbp.tile([128, N], BF, tag=f"hcs{p}")
                hct = zrps.tile([128, N], f32, tag="zr", name=f"hc{p}")
                zero = res is None or res.get("zero")
                for bi in range(2):
                    bs = slice(bi * 64, bi * 64 + 64)
                    b = 2 * p + bi
                    if K2X:
                        _, wxu_ap, x_ap = xa_ap(b, t)
                        nc.tensor.matmul(hct[bs, :], wxu_ap, x_ap,
                                         start=True, stop=zero,
                                         skip_group_check=(bi == 1))
                        if not zero:
                            nc.tensor.matmul(hct[bs, :], wu2[bs, :],
                                             res["k2S"][bs, :],
                                             start=False, stop=False)
                    else:
                        nc.tensor.matmul(hct[bs, :], wux2,
                                         res["ax2"][bi][:],
                                         start=True, stop=zero,
                                         skip_group_check=(bi == 1))
                    if not zero and not UPD_DR:
                        nc.tensor.matmul(hct[bs, :], wu0[bs, :],
                                         res["tAB"][0][bs, :], start=False,
                                         stop=False)
                for bi in range(2):
                    bs = slice(bi * 64, bi * 64 + 64)
                    if not zero:
                        if UPD_DR:
                            nc.tensor.matmul(
                                hct[bs, :], wu01[bs, :, :],
                                res["t01"][bs, :, :],
                                start=False, stop=True,
                                perf_mode=mybir.MatmulPerfMode.DoubleRow)
                        else:
                            nc.tensor.matmul(hct[bs, :], wu1[bs, :],
                                             res["tAB"][1][bs, :],
                                             start=False, stop=True)
                # both halves in one bank -> a single [128, 512] tanh
                nc.scalar.activation(hcs[:], hct[:], AF.Tanh, bias=bu2,
                                     scale=1.0 / SCL)
                st[p]["hcs"] = hcs

            def ph_update(p, t):
                zS, hcs = st[p]["zS"], st[p]["hcs"]
                hnew = sbp.tile([128, N], BF, tag=f"hw{p}")
                if h0_zero and t == 0:
                    # h1 = z * hc (since h == 0)
                    nc.vector.tensor_tensor(hnew[:], zS[:], hcs[:], ALU.mult)
                else:
                    u = st[p]["u"]
                    v = sbp.tile([128, N], BF, tag=f"tmp{p}", bufs=2)
                    if t < T - 1:
                        nc.vector.tensor_tensor(v[:], zS[:], hcs[:], ALU.mult)
                        nc.vector.tensor_tensor(hnew[:], u[:], v[:], ALU.add)
                    else:
                        # final step: halves, so the output DMA overlaps
                        for hh in range(2):
                            sl = slice(hh * (N // 2), (hh + 1) * (N // 2))
                            nc.vector.tensor_tensor(v[:, sl], zS[:, sl],
                                                    hcs[:, sl], ALU.mult)
                            nc.vector.tensor_tensor(hnew[:, sl], u[:, sl],
                                                    v[:, sl], ALU.add)
                            nc.sync.dma_start(HOUT_d[p, :, sl],
                                              hnew[:, sl])
                HW[p] = hnew
                if t < T - 1:
                    HN_s[p] = transp(p, hnew, "hn2", fp8=True)

            def dump(nm, ap):
                if debug:
                    nc.sync.dma_start(dbg[nm][0:ap.shape[0]], ap)

            def ph_dbg(p, t):
                import os
                if not debug or p != 0 or t != int(os.environ.get("DBG_T", "0")):
                    return
                dump("DXA", XA24[:])
                dump("DZS", st[0]["zS"][:])
                dump("DRS", st[0]["rS"][:])
                dump("DCH", st[0]["ch_dbg"][:])
                dump("DHCS", st[0]["hcs"][:])
                dump("DHW1", HW[0][:])
                dump("DHN", HN_s[0][:])
                dump("DCN", st[0]["CN"][:])

            PHASES = [ph_agg_gate, ph_gate_w, ph_rt, ph_agg_cand,
                      ph_upd_w, ph_update, ph_dbg]
            NPH = len(PHASES)
            OFF = OFF_TICKS
            xa_precompute()
            for tick in range(NPH * T + OFF):
                for p in range(2):
                    local = tick - OFF * p
                    if 0 <= local < NPH * T:
                        t, ph = divmod(local, NPH)
                        PHASES[ph](p, t)

    _split_excess_waits(nc, max_waits=1)
    return nc


_NC_CACHE = {}


def _get_nc(debug=False, h0_zero=False):
    key = f"nc{debug}{h0_zero}{CAND_FP8}{OFF_TICKS}{K2X}{GATE_DR}{UPD_DR}"
    if key not in _NC_CACHE:
        _NC_CACHE[key] = _build_nc(debug, h0_zero)
    return _NC_CACHE[key]


def _host_prep(G, x_seq, init_h, W_gate, b_gate, W_update, b_update):
    f32 = np.float32
    GTf = np.asarray(G, np.float32).transpose(0, 2, 1)         # [k, jn, i]
    GT = GTf.reshape(K, NT, 128, N).transpose(1, 2, 0, 3)      # [j, p, k, i]
    GT = np.ascontiguousarray(GT).reshape(NT, 128, K * N).astype(NPBF)
    # GT8[p, jp, k, kt, i] = fp8(64 * G_k[i, (2jp+kt)*128+p])
    G8 = (64.0 * GTf).reshape(K, NT // 2, 2, 128, N)           # [k, jp, kt, p, i]
    G8 = G8.transpose(3, 1, 0, 2, 4)                           # [p, jp, k, kt, i]
    GT8 = np.ascontiguousarray(G8).reshape(
        128, (NT // 2) * K * 2 * N).astype(NPF8)
    WG3 = np.asarray(W_gate, f32).reshape(K, P, 2 * H)
    WU3 = np.asarray(W_update, f32).reshape(K, P, H)
    WG0 = np.concatenate([WG3[0, C:, :]] * 2, axis=0)
    WG1 = np.concatenate([WG3[1, C:, :]] * 2, axis=0)
    WG2 = np.concatenate([WG3[2, C:, :]] * 2, axis=0)
    WU0 = np.concatenate([WU3[0, C:, :]] * 2, axis=0)
    WU1 = np.concatenate([WU3[1, C:, :]] * 2, axis=0)
    WU2 = np.concatenate([WU3[2, C:, :]] * 2, axis=0)
    # x-block rows (k,c): row k*2+c = W[k, c, :]
    xg = WG3[:, :C, :].reshape(K * C, 2 * H)
    xu = WU3[:, :C, :].reshape(K * C, H)
    WGX2 = np.concatenate([WG3[2, C:, :], xg], axis=0)
    WUX2 = np.concatenate([WU3[2, C:, :], xu], axis=0)
    S = SCL
    usc = 1024.0 if CAND_FP8 else 1.0
    WB = np.zeros((128, 898), f32)
    WB[:, 0:128] = S * WG0 / 1024.0
    WB[:, 128:256] = S * WG1 / 1024.0
    if K2X:
        WB[:, 256:384] = S * WG2 / 1024.0
        WB[:, 512:576] = S * WU2 / usc
        for b in range(BL):
            WB[b * 32:b * 32 + 6, 706:834] = S * xg
            WB[b * 32:b * 32 + 6, 834:898] = S * xu
    else:
        WB[0:70, 256:384] = S * WGX2
        WB[0:64, 256:384] = S * WGX2[0:64] / 1024.0
        WB[0:70, 512:576] = S * WUX2
        WB[0:64, 512:576] = S * WUX2[0:64] / usc
    WB[:, 384:448] = S * WU0 / usc
    WB[:, 448:512] = S * WU1 / usc
    WB[:, 576:704] = np.eye(128, dtype=f32)
    WB[:, 704] = S * np.asarray(b_gate, f32)
    WB[0:64, 705] = S * np.asarray(b_update, f32)
    WB[64:128, 705] = S * np.asarray(b_update, f32)
    # fp8 DoubleRow weight blob: product scale must come out at S
    # gate: t01g = (1024*agg) * (S/1024) -> w8 = W
    # update: t01u = agg * 16 (or 1024*agg * S/1024) -> w8 = W*4 (or W)
    WB8 = np.zeros((128, 384), f32)
    wgk = np.stack([WG3[0, C:, :], WG3[1, C:, :]], axis=1)    # [64,2,128]
    WB8[:, 0:256] = np.concatenate([wgk] * 2, axis=0).reshape(128, 256)
    s_wu = 1.0 if CAND_FP8 else 4.0
    wuk = np.stack([WU3[0, C:, :], WU3[1, C:, :]], axis=1) * s_wu
    WB8[:, 256:384] = np.concatenate([wuk] * 2, axis=0).reshape(128, 128)
    shared = {
        "GT": GT,
        "WB": WB.astype(NPBF),
        "WB8": WB8.astype(NPF8),
        "GT8": GT8,
    }
    x_seq = np.asarray(x_seq, f32)
    init_h = np.asarray(init_h, f32)
    in_maps = []
    for c in range(NCORES):
        b0 = c * BL
        xs = x_seq[b0:b0 + BL]                     # [4, 12, 512, 2]
        h0 = init_h[b0:b0 + BL]                    # [4, 512, 64]
        m = dict(shared)
        # XN cols (b, c, t)
        m["XN"] = np.ascontiguousarray(
            xs.transpose(2, 0, 3, 1)).reshape(N, BL * T * C).astype(NPBF)
        # HN0[p][n_loc, j*128 + b*64 + f] = h0[2p+b, j*128+n_loc, f]
        hn = h0.reshape(2, 2, NT, 128, H)          # [p, b, j, n, f]
        m["HN0"] = (16.0 * np.ascontiguousarray(
            hn.transpose(0, 3, 2, 1, 4)).reshape(2, 128, N)).astype(NPF8)
        # HW0[p][b*64 + f, i] = h0[2p+b, i, f]
        hw = h0.reshape(2, 2, N, H)                # [p, b, i, f]
        m["HW0"] = np.ascontiguousarray(
            hw.transpose(0, 1, 3, 2)).reshape(2, 128, N).astype(NPBF)
        in_maps.append(m)
    return in_maps


def _run(inputs, trace=False, debug=False):
    h0_zero = not np.any(np.asarray(inputs["init_h"]))
    nc = _get_nc(debug, h0_zero)
    in_maps = _host_prep(**inputs)
    res = run_bass_kernel_spmd(nc, in_maps, list(range(NCORES)), trace=trace)
    outs = []
    for c in range(NCORES):
        hout = np.asarray(res.results[c]["HOUT"], dtype=np.float32)
        # [2, 128, 512] -> [4, 512, 64]
        hout = hout.reshape(2, 2, H, N).transpose(0, 1, 3, 2).reshape(
            BL, N, H)
        outs.append(hout)
    full = np.concatenate(outs, axis=0).astype(np.float32)
    return full, res


def kernel(G, x_seq, init_h, W_gate, b_gate, W_update, b_update):
    full, _ = _run(dict(G=G, x_seq=x_seq, init_h=init_h, W_gate=W_gate,
                        b_gate=b_gate, W_update=W_update, b_update=b_update))
    return full
